# revision 28
# baseline (speedup 1.0000x reference)
"""BoundaryFluxAttention TRN2 kernel.

Distribution (8 cores): data-parallel over batch (B=2) x tensor-parallel over
heads (16 heads -> 4 groups of 4). Core c handles batch c//4, head group c%4.
Each core computes a partial output y_c = softmax-attention(its 4 heads) @ W_out
rows for those heads; the host sums the 4 partials per batch and adds b_out.

Per-core pipeline (T=2048, D=1024, 4 heads of hd=64), hand-pipelined emission:
  A:  QK^T projection qkt[db] [128, T] bf16 = (W slice)^T @ xT (fp32r matmuls);
      x arrives pre-transposed from the host. Scale hd^-0.5 folded into W_q/b_q.
  B:  V projection in natural [T, 256] layout -> vsb [128, kb, h, 65] bf16
      with a ones column at index 64 (denominator accumulates in the same
      matmul as O'^T).
  C:  S^T tiles [128k, 1024(2 heads)] = K_h^T.T @ Q_h^T, heads row-tiled
      (K=64 at partition offsets 0/64), bf16: ~113ns/matmul via PE row-group
      concurrency.
  exp: ScalarE, per-partition bias = boundary*0.1, bf16 out. This paces the
      CD loop (~1.1us per k-block) -> emission interleaves stage A/B and the
      previous group's epilogue so ACT never starves.
  D:  O'^T [65, 512] += V'_h.T @ P_h^T over k; row 64 = softmax denominator.
  norm: stage O' to SBUF (frees the PSUM accumulator fast), reciprocal of the
      denominator row, partition-broadcast via K=1 matmul, fused multiply.
      Odd heads shifted to partitions 64..127 via SBUF->SBUF DMA.
  E:  y = OT_pair @ W_out slice (fp32r), emitted per q-block as PE filler.
"""

import numpy as np

import concourse.bass as bass  # noqa: F401
import concourse.mybir as mybir
import concourse.tile as tile
from concourse import bacc

F32 = mybir.dt.float32
F32R = mybir.dt.float32r
BF16 = mybir.dt.bfloat16
EXP = mybir.ActivationFunctionType.Exp
FP16 = mybir.dt.float16

T = 2048
D = 1024
HPC = 4          # heads per core
HD = 64
NKB = T // 128   # 16 k/t blocks of 128
NQB = T // 512   # 4 q blocks of 512
NCH = D // 128   # 8 contraction chunks
SCALE = HD ** -0.5
BIAS_COEF = 0.1

_NC_CACHE = {}


def _ensure_patched_act_root():
    """Point walrus at an act_info.json with natural_log_exp_and_others
    listed first, so the kernel's Exp and Ln activations resolve to one
    table set (the default greedy order ping-pongs between exp_and_others
    and natural_log, costing a ~2.7us ACT_TABLE_LOAD per switch)."""
    import json
    import os
    import tempfile

    if os.environ.get("BASS_ACT_ROOT_JSON_PATH"):
        return
    try:
        from neuronxcc.driver.Job import Job
        from neuronxcc.driver.jobs.support.FindActInfo import findActInfoFile

        src_json = findActInfoFile(Job.getPackageDir(), "gen3")
    except Exception:
        return
    try:
        d = os.path.dirname(src_json)
        out = tempfile.mkdtemp(prefix="act_root_")
        for f in os.listdir(d):
            os.symlink(os.path.join(d, f), os.path.join(out, f))
        with open(src_json) as fh:
            ai = json.load(fh)
        sets = ai.get("act_func_sets", [])
        nle = [s for s in sets if s["name"] == "natural_log_exp_and_others"]
        if not nle:
            return
        ai["act_func_sets"] = nle + [
            s for s in sets if s["name"] != "natural_log_exp_and_others"
        ]
        os.remove(os.path.join(out, "act_info.json"))
        with open(os.path.join(out, "act_info.json"), "w") as fh:
            json.dump(ai, fh)
        os.environ["BASS_ACT_ROOT_JSON_PATH"] = os.path.join(out, "act_info.json")
    except Exception:
        pass



def _build_nc(with_qkv_bias=True):
    nc = bacc.Bacc("TRN2", target_bir_lowering=False)

    xt_d = nc.declare_dram_parameter("xt", [D, T], BF16, isOutput=False)
    wqk_d = nc.declare_dram_parameter("wqk", [D, 512], BF16, isOutput=False)
    bqk_d = nc.declare_dram_parameter("bqk", [1, 512], F32R, isOutput=False)
    wv_d = nc.declare_dram_parameter("wv", [D, 256], BF16, isOutput=False)
    bv_d = nc.declare_dram_parameter("bv", [1, 256], F32R, isOutput=False)
    wo_d = nc.declare_dram_parameter("wo", [256, D], F32R, isOutput=False)
    bs_d = nc.declare_dram_parameter("bs", [128, NKB], F32, isOutput=False)
    ones_d = nc.declare_dram_parameter("ones", [1, 512], F32R, isOutput=False)
    ones128_d = nc.declare_dram_parameter("ones128", [128, 64], F32R, isOutput=False)
    y_d = nc.declare_dram_parameter("y", [T, D], BF16, isOutput=True)

    with tile.TileContext(nc) as tc:
        with (
            tc.tile_pool(name="const", bufs=1) as constp,
            tc.tile_pool(name="wts", bufs=1) as wts,
            tc.tile_pool(name="big", bufs=1) as bigp,
            tc.tile_pool(name="pt", bufs=4) as ptp,
            tc.tile_pool(name="norm", bufs=1) as normp,
            tc.tile_pool(name="stg", bufs=2) as stgp,
            tc.tile_pool(name="ysb", bufs=3) as ypool,
            tc.tile_pool(name="psG", bufs=2, space="PSUM") as psG,
            tc.tile_pool(name="psS", bufs=2, space="PSUM") as psS,
            tc.tile_pool(name="psO", bufs=1, space="PSUM") as psO,
        ):
            # ---------------- constants / weights (scalar DMA queue) --------
            # x^T first, one big DMA per t-group on the sync queue: a single
            # DMA_DIRECT2D fans its packets across all 16 HW DMA engines, so
            # issuing 4 instead of 32 removes ~18us of SP-side issue
            # serialization and keeps arrival strictly t-group-ordered.
            xT = bigp.tile([128, NCH, T], BF16, tag="xT")
            xt_pct = xt_d.rearrange("(c p) t -> p c t", p=128)
            for tb in range(4):
                nc.sync.dma_start(
                    xT[:, :, tb * 512:(tb + 1) * 512],
                    xt_pct[:, :, tb * 512:(tb + 1) * 512],
                )

            wqk_sb = wts.tile([128, NCH, 512], BF16, tag="wqk")
            nc.scalar.dma_start(wqk_sb[:], wqk_d.rearrange("(c p) n -> p c n", p=128))
            bqk_sb = wts.tile([1, 512], F32R, tag="bqk")
            nc.scalar.dma_start(bqk_sb[:], bqk_d[:])
            wv_sb = wts.tile([128, NCH, 256], BF16, tag="wv")
            nc.scalar.dma_start(wv_sb[:], wv_d.rearrange("(c p) n -> p c n", p=128))
            bv_sb = wts.tile([1, 256], F32R, tag="bv")
            nc.scalar.dma_start(bv_sb[:], bv_d[:])

            ones = constp.tile([1, 512], F32R, tag="ones")
            nc.scalar.dma_start(ones[:], ones_d[:])
            ones128 = constp.tile([128, 64], F32R, tag="ones128")
            nc.scalar.dma_start(ones128[:], ones128_d[:])
            bs_sb = constp.tile([128, NKB], F32, tag="bs")
            nc.scalar.dma_start(bs_sb[:], bs_d[:])

            wo_sb = wts.tile([128, 2, D], F32R, tag="wo")
            nc.scalar.dma_start(wo_sb[:], wo_d.rearrange("(c p) n -> p c n", p=128))

            qkt = [
                bigp.tile([128, T], BF16, tag=f"qkt{db}", name=f"qkt{db}")
                for db in range(4)
            ]
            vsb = bigp.tile([128, NKB, HPC, 65], BF16, tag="vsb", name="vsb_v10")
            nc.gpsimd.memset(vsb[:], 1.0)
            ot = [
                bigp.tile([128, T], F32R, tag=f"ot{pi}", name=f"ot{pi}")
                for pi in range(2)
            ]
            y_rows = y_d.rearrange("(n p) d -> n p d", p=128)

            # ---------------- emission helpers ----------------
            def emit_A(tb):
                for db in range(4):
                    ps = psG.tile([128, 512], F32, tag="gp", name=f"qk{tb}_{db}")
                    for c in range(NCH):
                        nc.tensor.matmul(
                            ps[:],
                            wqk_sb[:, c, db * 128:(db + 1) * 128],
                            xT[:, c, tb * 512:(tb + 1) * 512],
                            start=(c == 0),
                            stop=(not with_qkv_bias and c == NCH - 1),
                        )
                    if with_qkv_bias:
                        nc.tensor.matmul(
                            ps[:],
                            bqk_sb[0:1, db * 128:(db + 1) * 128],
                            ones[0:1, :],
                            start=False,
                            stop=True,
                        )
                    nc.vector.tensor_copy(qkt[db][:, tb * 512:(tb + 1) * 512], ps[:])

            def emit_B(tb):
                for j in range(4):
                    kb = tb * 4 + j
                    ps = psG.tile([128, 256], F32, tag="gp", name=f"v{kb}")
                    for c in range(NCH):
                        nc.tensor.matmul(
                            ps[:],
                            xT[:, c, kb * 128:(kb + 1) * 128],
                            wv_sb[:, c, :],
                            start=(c == 0),
                            stop=(not with_qkv_bias and c == NCH - 1),
                        )
                    if with_qkv_bias:
                        nc.tensor.matmul(
                            ps[:], ones[0:1, 0:128], bv_sb[:], start=False, stop=True
                        )
                    nc.vector.tensor_copy(
                        vsb[:, kb, :, 0:64],
                        ps[:].rearrange("p (h c) -> p h c", h=HPC),
                    )

            def emit_S_exp(qb, pi, kb):
                qdb, kdb = pi, 2 + pi
                s01 = psS.tile([128, 1024], F32, tag="s01", name=f"s{qb}_{pi}_{kb}")
                nc.tensor.matmul(
                    s01[:, 0:512],
                    qkt[kdb][0:64, kb * 128:(kb + 1) * 128],
                    qkt[qdb][0:64, qb * 512:(qb + 1) * 512],
                )
                nc.tensor.matmul(
                    s01[:, 512:1024],
                    qkt[kdb][64:128, kb * 128:(kb + 1) * 128],
                    qkt[qdb][64:128, qb * 512:(qb + 1) * 512],
                )
                p01 = ptp.tile([128, 1024], BF16, tag="p01", name=f"p{qb}_{pi}_{kb}")
                nc.scalar.activation(p01[:], s01[:], EXP, bias=bs_sb[:, kb:kb + 1])
                return p01

            def emit_D(qb, pi, kb, p01, osA, osB):
                nc.tensor.matmul(
                    osA[:], vsb[:, kb, 2 * pi, :], p01[:, 0:512],
                    start=(kb == 0), stop=(kb == NKB - 1),
                )
                nc.tensor.matmul(
                    osB[:], vsb[:, kb, 2 * pi + 1, :], p01[:, 512:1024],
                    start=(kb == 0), stop=(kb == NKB - 1),
                )

            stgq_tiles = {}

            def emit_stage_out(qb, pi, osA, osB):
                # Stage O' out of PSUM immediately so the accumulator banks
                # free for the next group; normalization is deferred and
                # batched per q-block (emit_norm_qb).
                if pi == 0:
                    stgq_tiles[qb] = stgp.tile(
                        [65, 4, 512], F32R, tag="stgq", name=f"stgq{qb}"
                    )
                stgq = stgq_tiles[qb]
                nc.vector.tensor_copy(stgq[:, 2 * pi + 0, :], osA[:])
                nc.vector.tensor_copy(stgq[:, 2 * pi + 1, :], osB[:])

            norm_aux = {}

            def make_norm_half(qb, pi):
                # Normalization for one head pair (pi), runnable one group
                # earlier than a whole-q-block norm: fan the pair's raw
                # denominator rows onto partitions 64*pi/64*pi+32 via tiny
                # SBUF DMAs, one lane-parallel DVE reciprocal over that
                # 33-partition band, then per-head K=1 rebroadcast of the
                # f32r reciprocal + fused normalize-multiply (+ odd-head
                # SBUF shift). ScalarE stays exp-only throughout.
                stgq = stgq_tiles[qb]
                cols = slice(qb * 512, (qb + 1) * 512)
                if pi == 0:
                    dt = normp.tile([128, 512], F32R, tag="dt", name=f"dt{qb}")
                    rec = normp.tile(
                        [128, 512], F32R, tag="rec", name=f"rec{qb}"
                    )
                    norm_aux[qb] = (dt, rec)
                else:
                    dt, rec = norm_aux.pop(qb)
                    stgq_tiles.pop(qb)

                def s_dma(stgq=stgq, dt=dt, pi=pi):
                    if pi == 0:
                        nc.gpsimd.memset(dt[:].bitcast(F32), 1.0)
                    for hj in range(2):
                        j = 2 * pi + hj
                        eng = nc.gpsimd if hj == 0 else nc.sync
                        eng.dma_start(
                            dt[32 * j:32 * j + 1, :], stgq[64:65, j, :]
                        )

                def s_rec(dt=dt, rec=rec, pi=pi):
                    lo = 64 * pi
                    with nc.allow_low_precision(reason="f32r recip broadcast"):
                        nc.vector.reciprocal(
                            rec[lo:lo + 33, :].opt(), dt[lo:lo + 33, :].opt()
                        )

                steps = [s_dma, s_rec]
                for hj in range(2):
                    def s_j(qb=qb, stgq=stgq, rec=rec, pi=pi, hj=hj):
                        j = 2 * pi + hj
                        bc2 = psG.tile(
                            [64, 512], F32, tag="gp", name=f"bc2_{qb}_{j}"
                        )
                        nc.tensor.matmul(
                            bc2[:], ones128[32 * j:32 * j + 1, 0:64],
                            rec[32 * j:32 * j + 1, :],
                            tile_position=(32 * j, 0),
                        )
                        if hj == 0:
                            nc.vector.tensor_mul(
                                ot[pi][0:64, cols], stgq[0:64, j, :], bc2[0:64, :]
                            )
                        else:
                            stag = normp.tile([64, 512], F32R, tag="stag")
                            nc.vector.tensor_mul(
                                stag[:], stgq[0:64, j, :], bc2[0:64, :]
                            )
                            nc.gpsimd.dma_start(ot[pi][64:128, cols], stag[:])
                    steps.append(s_j)
                return steps

            def make_E_chunks(qb, tail=False):
                # Stage E for one q-block, sliced into 16 small closures so the
                # emission can interleave them per k-iteration of the next
                # attention group (keeps PE dense without starving ScalarE).
                # In the tail there is no next group: accumulate in psS-backed
                # PSUM instead of psG so chunks can interleave with the final
                # norm's bc2 tiles without WAR deadlock.
                chunks = []
                for j in range(4):
                    tb = qb * 4 + j
                    state = {}

                    def c0(tb=tb, state=state):
                        state["ysb"] = ypool.tile(
                            [128, D], BF16, tag="ysb", name=f"ysb{tb}"
                        )
                        if tail:
                            pair = psS.tile(
                                [128, 2, 512], F32, tag="s01", name=f"yp{tb}"
                            )
                            state["yps"] = [pair[:, 0, :], pair[:, 1, :]]
                        else:
                            state["yps"] = [
                                psG.tile(
                                    [128, 512], F32, tag="gp",
                                    name=f"yps{tb}_{nb}",
                                )
                                for nb in range(2)
                            ]
                        nc.tensor.matmul(
                            state["yps"][0],
                            ot[0][:, tb * 128:(tb + 1) * 128],
                            wo_sb[:, 0, 0:512],
                            start=True, stop=False,
                        )

                    def c1(tb=tb, state=state):
                        nc.tensor.matmul(
                            state["yps"][1],
                            ot[0][:, tb * 128:(tb + 1) * 128],
                            wo_sb[:, 0, 512:1024],
                            start=True, stop=False,
                        )

                    def c2(tb=tb, state=state):
                        nc.tensor.matmul(
                            state["yps"][0],
                            ot[1][:, tb * 128:(tb + 1) * 128],
                            wo_sb[:, 1, 0:512],
                            start=False, stop=True,
                        )
                        nc.vector.tensor_copy(
                            state["ysb"][:, 0:512], state["yps"][0]
                        )

                    def c3(tb=tb, state=state):
                        nc.tensor.matmul(
                            state["yps"][1],
                            ot[1][:, tb * 128:(tb + 1) * 128],
                            wo_sb[:, 1, 512:1024],
                            start=False, stop=True,
                        )
                        nc.vector.tensor_copy(
                            state["ysb"][:, 512:1024], state["yps"][1]
                        )
                        nc.sync.dma_start(y_rows[tb], state["ysb"][:])

                    chunks += [c0, c1, c2, c3]
                return chunks

            def emit_E(qb):
                for ch in make_E_chunks(qb):
                    ch()

            # ---------------- pipelined emission ----------------
            # Phase 1: stages A/B per t-group, with CD(q0, pair0) k-iterations
            # interleaved so ScalarE ramps while the PE grinds projections.
            osA = psO.tile([65, 512], F32, tag="osA", name="osA0_0")
            osB = psO.tile([65, 512], F32, tag="osB", name="osB0_0")
            for tb in range(4):
                emit_A(tb)
                emit_B(tb)
                for kb in range(4 * tb, 4 * tb + 4):
                    p01 = emit_S_exp(0, 0, kb)
                    emit_D(0, 0, kb, p01, osA, osB)
            pending = [(0, 0, osA, osB)]

            # Phase 2: remaining groups; each group's first two S/exp pairs
            # are emitted before the previous group's epilogue so ACT stays fed
            # across the boundary. Norm steps and E chunks of the previous
            # q-block are slotted at fixed k-iterations so their PE pieces
            # never head-of-line-block on DVE results.
            groups = [(0, 1)] + [(qb, pi) for qb in range(1, NQB) for pi in range(2)]
            e_chunks = []
            for qb, pi in groups:
                head = [emit_S_exp(qb, pi, kb) for kb in (0, 1)]
                pqb, ppi, posA, posB = pending.pop()
                emit_stage_out(pqb, ppi, posA, posB)
                norm_steps = make_norm_half(pqb, ppi)
                if ppi == 1:
                    e_chunks = make_E_chunks(pqb)
                osA = psO.tile([65, 512], F32, tag="osA", name=f"osA{qb}_{pi}")
                osB = psO.tile([65, 512], F32, tag="osB", name=f"osB{qb}_{pi}")
                for kb in (0, 1):
                    emit_D(qb, pi, kb, head[kb], osA, osB)
                for kb in range(2, NKB):
                    p01 = emit_S_exp(qb, pi, kb)
                    emit_D(qb, pi, kb, p01, osA, osB)
                    if norm_steps and kb in (2, 3, 5, 6):
                        norm_steps.pop(0)()
                    elif e_chunks and kb >= 8:
                        e_chunks.pop(0)()
                        if e_chunks and kb >= 9:
                            e_chunks.pop(0)()
                pending = [(qb, pi, osA, osB)]

            # Tail: last group's second-half norm + its E stage. E accumulates
            # in psS-backed PSUM so chunks interleave with bc2 allocations.
            qb, pi, osA, osB = pending.pop()
            emit_stage_out(qb, pi, osA, osB)
            s_dma, s_rec, s_2, s_3 = make_norm_half(qb, pi)
            tail_E = make_E_chunks(qb, tail=True)
            tbs = [tail_E[4 * t:4 * t + 4] for t in range(4)]
            s_dma()
            tbs[0][0](); tbs[0][1]()              # c0,c1 of tb0 (ot[0] ready)
            s_rec()
            tbs[1][0](); tbs[1][1]()              # fills part of recip latency
            s_2()
            s_3()
            tbs[0][2](); tbs[0][3]()
            tbs[1][2](); tbs[1][3]()
            for t in (2, 3):
                for c in tbs[t]:
                    c()
    nc.compile()
    return nc


def _get_nc(with_qkv_bias=True):
    key = ("nc", with_qkv_bias)
    if key not in _NC_CACHE:
        _NC_CACHE[key] = _build_nc(with_qkv_bias)
    return _NC_CACHE[key]


def _make_in_maps(x, boundary_score, W_qkv, b_qkv, W_out):
    x = np.asarray(x, np.float32)
    boundary_score = np.asarray(boundary_score, np.float32)
    W_qkv = np.asarray(W_qkv, np.float32)
    b_qkv = np.asarray(b_qkv, np.float32)
    W_out = np.asarray(W_out, np.float32)

    Wq, Wk, Wv = W_qkv[:, :D], W_qkv[:, D:2 * D], W_qkv[:, 2 * D:]
    bq, bk, bv = b_qkv[:D], b_qkv[D:2 * D], b_qkv[2 * D:]
    ones = np.ones((1, 512), np.float32)
    ones128 = np.ones((128, 64), np.float32)
    import ml_dtypes
    bf16 = ml_dtypes.bfloat16
    xts = [np.ascontiguousarray(x[b].T).astype(bf16) for b in range(x.shape[0])]

    in_maps = []
    for c in range(8):
        b, g = divmod(c, 4)
        lo, hi = 256 * g, 256 * (g + 1)
        wqk = np.ascontiguousarray(
            np.concatenate([Wq[:, lo:hi] * SCALE, Wk[:, lo:hi]], axis=1)
        ).astype(bf16)
        bqk = np.concatenate([bq[lo:hi] * SCALE, bk[lo:hi]])[None]
        wv = np.ascontiguousarray(Wv[:, lo:hi]).astype(bf16)
        bvv = np.ascontiguousarray(bv[lo:hi][None])
        wo = np.ascontiguousarray(W_out[lo:hi, :])
        bs = np.ascontiguousarray(
            (boundary_score[b] * BIAS_COEF).reshape(NKB, 128).T
        )
        in_maps.append(
            dict(
                xt=xts[b], wqk=wqk, bqk=np.ascontiguousarray(bqk),
                wv=wv, bv=bvv, wo=wo, bs=bs, ones=ones, ones128=ones128,
            )
        )
    return in_maps


def kernel(x, boundary_score, W_qkv, b_qkv, W_out, b_out):
    from concourse.bass_utils import run_bass_kernel_spmd

    x = np.asarray(x, np.float32)
    B = x.shape[0]
    in_maps = _make_in_maps(x, boundary_score, W_qkv, b_qkv, W_out)
    nc = _get_nc(with_qkv_bias=bool(np.any(np.asarray(b_qkv))))
    res = run_bass_kernel_spmd(nc, in_maps, list(range(8))).results
    out = np.zeros((B, T, D), np.float32)
    for c in range(8):
        out[c // 4] += np.asarray(res[c]["y"], np.float32)
    out += np.asarray(b_out, np.float32)
    return out



# revision 29
# speedup vs baseline: 1.0007x; 1.0007x over previous
"""BoundaryFluxAttention TRN2 kernel.

Distribution (8 cores): data-parallel over batch (B=2) x tensor-parallel over
heads (16 heads -> 4 groups of 4). Core c handles batch c//4, head group c%4.
Each core computes a partial output y_c = softmax-attention(its 4 heads) @ W_out
rows for those heads; the host sums the 4 partials per batch and adds b_out.

Per-core pipeline (T=2048, D=1024, 4 heads of hd=64), hand-pipelined emission:
  A:  QK^T projection qkt[db] [128, T] bf16 = (W slice)^T @ xT (fp32r matmuls);
      x arrives pre-transposed from the host. Scale hd^-0.5 folded into W_q/b_q.
  B:  V projection in natural [T, 256] layout -> vsb [128, kb, h, 65] bf16
      with a ones column at index 64 (denominator accumulates in the same
      matmul as O'^T).
  C:  S^T tiles [128k, 1024(2 heads)] = K_h^T.T @ Q_h^T, heads row-tiled
      (K=64 at partition offsets 0/64), bf16: ~113ns/matmul via PE row-group
      concurrency.
  exp: ScalarE, per-partition bias = boundary*0.1, bf16 out. This paces the
      CD loop (~1.1us per k-block) -> emission interleaves stage A/B and the
      previous group's epilogue so ACT never starves.
  D:  O'^T [65, 512] += V'_h.T @ P_h^T over k; row 64 = softmax denominator.
  norm: stage O' to SBUF (frees the PSUM accumulator fast), reciprocal of the
      denominator row, partition-broadcast via K=1 matmul, fused multiply.
      Odd heads shifted to partitions 64..127 via SBUF->SBUF DMA.
  E:  y = OT_pair @ W_out slice (fp32r), emitted per q-block as PE filler.
"""

import numpy as np

import concourse.bass as bass  # noqa: F401
import concourse.mybir as mybir
import concourse.tile as tile
from concourse import bacc

F32 = mybir.dt.float32
F32R = mybir.dt.float32r
BF16 = mybir.dt.bfloat16
EXP = mybir.ActivationFunctionType.Exp
FP16 = mybir.dt.float16

T = 2048
D = 1024
HPC = 4          # heads per core
HD = 64
NKB = T // 128   # 16 k/t blocks of 128
NQB = T // 512   # 4 q blocks of 512
NCH = D // 128   # 8 contraction chunks
SCALE = HD ** -0.5
BIAS_COEF = 0.1

_NC_CACHE = {}


def _ensure_patched_act_root():
    """Point walrus at an act_info.json with natural_log_exp_and_others
    listed first, so the kernel's Exp and Ln activations resolve to one
    table set (the default greedy order ping-pongs between exp_and_others
    and natural_log, costing a ~2.7us ACT_TABLE_LOAD per switch)."""
    import json
    import os
    import tempfile

    if os.environ.get("BASS_ACT_ROOT_JSON_PATH"):
        return
    try:
        from neuronxcc.driver.Job import Job
        from neuronxcc.driver.jobs.support.FindActInfo import findActInfoFile

        src_json = findActInfoFile(Job.getPackageDir(), "gen3")
    except Exception:
        return
    try:
        d = os.path.dirname(src_json)
        out = tempfile.mkdtemp(prefix="act_root_")
        for f in os.listdir(d):
            os.symlink(os.path.join(d, f), os.path.join(out, f))
        with open(src_json) as fh:
            ai = json.load(fh)
        sets = ai.get("act_func_sets", [])
        nle = [s for s in sets if s["name"] == "natural_log_exp_and_others"]
        if not nle:
            return
        ai["act_func_sets"] = nle + [
            s for s in sets if s["name"] != "natural_log_exp_and_others"
        ]
        os.remove(os.path.join(out, "act_info.json"))
        with open(os.path.join(out, "act_info.json"), "w") as fh:
            json.dump(ai, fh)
        os.environ["BASS_ACT_ROOT_JSON_PATH"] = os.path.join(out, "act_info.json")
    except Exception:
        pass



def _build_nc(with_qkv_bias=True):
    nc = bacc.Bacc("TRN2", target_bir_lowering=False)

    xt_d = nc.declare_dram_parameter("xt", [D, T], BF16, isOutput=False)
    wqk_d = nc.declare_dram_parameter("wqk", [D, 512], BF16, isOutput=False)
    bqk_d = nc.declare_dram_parameter("bqk", [1, 512], F32R, isOutput=False)
    wv_d = nc.declare_dram_parameter("wv", [D, 256], BF16, isOutput=False)
    bv_d = nc.declare_dram_parameter("bv", [1, 256], F32R, isOutput=False)
    wo_d = nc.declare_dram_parameter("wo", [256, D], F32R, isOutput=False)
    bs_d = nc.declare_dram_parameter("bs", [128, NKB], F32, isOutput=False)
    ones_d = nc.declare_dram_parameter("ones", [1, 512], F32R, isOutput=False)
    ones128_d = nc.declare_dram_parameter("ones128", [128, 64], F32R, isOutput=False)
    y_d = nc.declare_dram_parameter("y", [T, D], BF16, isOutput=True)

    with tile.TileContext(nc) as tc:
        with (
            tc.tile_pool(name="const", bufs=1) as constp,
            tc.tile_pool(name="wts", bufs=1) as wts,
            tc.tile_pool(name="big", bufs=1) as bigp,
            tc.tile_pool(name="pt", bufs=4) as ptp,
            tc.tile_pool(name="norm", bufs=1) as normp,
            tc.tile_pool(name="stg", bufs=2) as stgp,
            tc.tile_pool(name="ysb", bufs=3) as ypool,
            tc.tile_pool(name="psG", bufs=2, space="PSUM") as psG,
            tc.tile_pool(name="psS", bufs=2, space="PSUM") as psS,
            tc.tile_pool(name="psO", bufs=1, space="PSUM") as psO,
        ):
            # ---------------- constants / weights (scalar DMA queue) --------
            # x^T first, one big DMA per t-group on the sync queue: a single
            # DMA_DIRECT2D fans its packets across all 16 HW DMA engines, so
            # issuing 4 instead of 32 removes ~18us of SP-side issue
            # serialization and keeps arrival strictly t-group-ordered.
            xT = bigp.tile([128, NCH, T], BF16, tag="xT")
            xt_pct = xt_d.rearrange("(c p) t -> p c t", p=128)
            for tb in range(4):
                nc.sync.dma_start(
                    xT[:, :, tb * 512:(tb + 1) * 512],
                    xt_pct[:, :, tb * 512:(tb + 1) * 512],
                )

            wqk_sb = wts.tile([128, NCH, 512], BF16, tag="wqk")
            nc.scalar.dma_start(wqk_sb[:], wqk_d.rearrange("(c p) n -> p c n", p=128))
            bqk_sb = wts.tile([1, 512], F32R, tag="bqk")
            nc.scalar.dma_start(bqk_sb[:], bqk_d[:])
            wv_sb = wts.tile([128, NCH, 256], BF16, tag="wv")
            nc.scalar.dma_start(wv_sb[:], wv_d.rearrange("(c p) n -> p c n", p=128))
            bv_sb = wts.tile([1, 256], F32R, tag="bv")
            nc.scalar.dma_start(bv_sb[:], bv_d[:])

            ones = constp.tile([1, 512], F32R, tag="ones")
            nc.scalar.dma_start(ones[:], ones_d[:])
            ones128 = constp.tile([128, 64], F32R, tag="ones128")
            nc.scalar.dma_start(ones128[:], ones128_d[:])
            bs_sb = constp.tile([128, NKB], F32, tag="bs")
            nc.scalar.dma_start(bs_sb[:], bs_d[:])

            wo_sb = wts.tile([128, 2, D], F32R, tag="wo")
            nc.scalar.dma_start(wo_sb[:], wo_d.rearrange("(c p) n -> p c n", p=128))

            qkt = [
                bigp.tile([128, T], BF16, tag=f"qkt{db}", name=f"qkt{db}")
                for db in range(4)
            ]
            vsb = bigp.tile([128, NKB, HPC, 65], BF16, tag="vsb", name="vsb_v10")
            nc.gpsimd.memset(vsb[:], 1.0)
            ot = [
                bigp.tile([128, T], F32R, tag=f"ot{pi}", name=f"ot{pi}")
                for pi in range(2)
            ]
            y_rows = y_d.rearrange("(n p) d -> n p d", p=128)

            # ---------------- emission helpers ----------------
            def emit_A(tb):
                for db in range(4):
                    ps = psG.tile([128, 512], F32, tag="gp", name=f"qk{tb}_{db}")
                    for c in range(NCH):
                        nc.tensor.matmul(
                            ps[:],
                            wqk_sb[:, c, db * 128:(db + 1) * 128],
                            xT[:, c, tb * 512:(tb + 1) * 512],
                            start=(c == 0),
                            stop=(not with_qkv_bias and c == NCH - 1),
                        )
                    if with_qkv_bias:
                        nc.tensor.matmul(
                            ps[:],
                            bqk_sb[0:1, db * 128:(db + 1) * 128],
                            ones[0:1, :],
                            start=False,
                            stop=True,
                        )
                    nc.vector.tensor_copy(qkt[db][:, tb * 512:(tb + 1) * 512], ps[:])

            def emit_B(tb):
                for j in range(4):
                    kb = tb * 4 + j
                    ps = psG.tile([128, 256], F32, tag="gp", name=f"v{kb}")
                    for c in range(NCH):
                        nc.tensor.matmul(
                            ps[:],
                            xT[:, c, kb * 128:(kb + 1) * 128],
                            wv_sb[:, c, :],
                            start=(c == 0),
                            stop=(not with_qkv_bias and c == NCH - 1),
                        )
                    if with_qkv_bias:
                        nc.tensor.matmul(
                            ps[:], ones[0:1, 0:128], bv_sb[:], start=False, stop=True
                        )
                    nc.vector.tensor_copy(
                        vsb[:, kb, :, 0:64],
                        ps[:].rearrange("p (h c) -> p h c", h=HPC),
                    )

            def emit_S_exp(qb, pi, kb):
                qdb, kdb = pi, 2 + pi
                s01 = psS.tile([128, 1024], F32, tag="s01", name=f"s{qb}_{pi}_{kb}")
                nc.tensor.matmul(
                    s01[:, 0:512],
                    qkt[kdb][0:64, kb * 128:(kb + 1) * 128],
                    qkt[qdb][0:64, qb * 512:(qb + 1) * 512],
                )
                nc.tensor.matmul(
                    s01[:, 512:1024],
                    qkt[kdb][64:128, kb * 128:(kb + 1) * 128],
                    qkt[qdb][64:128, qb * 512:(qb + 1) * 512],
                )
                p01 = ptp.tile([128, 1024], BF16, tag="p01", name=f"p{qb}_{pi}_{kb}")
                nc.scalar.activation(p01[:], s01[:], EXP, bias=bs_sb[:, kb:kb + 1])
                return p01

            def emit_D(qb, pi, kb, p01, osA, osB):
                nc.tensor.matmul(
                    osA[:], vsb[:, kb, 2 * pi, :], p01[:, 0:512],
                    start=(kb == 0), stop=(kb == NKB - 1),
                )
                nc.tensor.matmul(
                    osB[:], vsb[:, kb, 2 * pi + 1, :], p01[:, 512:1024],
                    start=(kb == 0), stop=(kb == NKB - 1),
                )

            stgq_tiles = {}

            def emit_stage_out(qb, pi, osA, osB):
                # Stage O' out of PSUM immediately so the accumulator banks
                # free for the next group; normalization is deferred and
                # batched per q-block (emit_norm_qb).
                if pi == 0:
                    stgq_tiles[qb] = stgp.tile(
                        [65, 4, 512], F32R, tag="stgq", name=f"stgq{qb}"
                    )
                stgq = stgq_tiles[qb]
                nc.vector.tensor_copy(stgq[:, 2 * pi + 0, :], osA[:])
                nc.vector.tensor_copy(stgq[:, 2 * pi + 1, :], osB[:])

            def make_norm_steps(qb):
                # Normalization as schedulable steps so the PE never waits:
                #  step 0: 4 K=1 matmuls fan each head's raw denominator row
                #          into a 32-partition quadrant of one [128,512] PSUM
                #          tile, then ONE exact DVE reciprocal over all 128
                #          partitions (lane-parallel: ~3.2us, vs 12.9us on a
                #          single-partition row). ScalarE stays exp-only.
                #  steps 1..4: per-head K=1 rebroadcast of the f32r reciprocal
                #          + fused normalize-multiply (+ odd-head SBUF shift).
                stgq = stgq_tiles.pop(qb)
                cols = slice(qb * 512, (qb + 1) * 512)
                dt = normp.tile([128, 512], F32R, tag="dt", name=f"dt{qb}")
                rec = normp.tile([128, 512], F32R, tag="rec", name=f"rec{qb}")

                def s_dma(stgq=stgq, dt=dt):
                    nc.gpsimd.memset(dt[:].bitcast(F32), 1.0)
                    # Fan the 4 heads' denominator rows onto partitions
                    # 0/32/64/96 (tiny partition-moving SBUF DMAs) so ONE
                    # [128,512] DVE reciprocal covers all heads lane-parallel
                    # (~3.2us vs 12.9us on a single-partition row).
                    for j in range(4):
                        eng = nc.gpsimd if j % 2 == 0 else nc.sync
                        eng.dma_start(
                            dt[32 * j:32 * j + 1, :], stgq[64:65, j, :]
                        )

                def s_rec(dt=dt, rec=rec):
                    with nc.allow_low_precision(reason="f32r recip broadcast"):
                        nc.vector.reciprocal(
                            rec[0:97, :].opt(), dt[0:97, :].opt()
                        )

                steps = [s_dma, s_rec]
                for j in range(4):
                    def s_j(qb=qb, stgq=stgq, rec=rec, j=j):
                        pi, parity = divmod(j, 2)
                        bc2 = psG.tile(
                            [64, 512], F32, tag="gp", name=f"bc2_{qb}_{j}"
                        )
                        nc.tensor.matmul(
                            bc2[:], ones128[32 * j:32 * j + 1, 0:64],
                            rec[32 * j:32 * j + 1, :],
                            tile_position=(32 * j, 0),
                        )
                        if parity == 0:
                            nc.vector.tensor_mul(
                                ot[pi][0:64, cols], stgq[0:64, j, :], bc2[0:64, :]
                            )
                        else:
                            stag = normp.tile([64, 512], F32R, tag="stag")
                            nc.vector.tensor_mul(
                                stag[:], stgq[0:64, j, :], bc2[0:64, :]
                            )
                            nc.gpsimd.dma_start(ot[pi][64:128, cols], stag[:])
                    steps.append(s_j)
                return steps

            def make_E_chunks(qb):
                # Stage E for one q-block, sliced into 16 small closures so the
                # emission can interleave one chunk per k-iteration of the next
                # attention group (keeps PE dense without starving ScalarE).
                chunks = []
                for j in range(4):
                    tb = qb * 4 + j
                    state = {}

                    def c0(tb=tb, state=state):
                        state["ysb"] = ypool.tile(
                            [128, D], BF16, tag="ysb", name=f"ysb{tb}"
                        )
                        state["yps"] = [
                            psG.tile([128, 512], F32, tag="gp", name=f"yps{tb}_{nb}")
                            for nb in range(2)
                        ]
                        nc.tensor.matmul(
                            state["yps"][0][:],
                            ot[0][:, tb * 128:(tb + 1) * 128],
                            wo_sb[:, 0, 0:512],
                            start=True, stop=False,
                        )

                    def c1(tb=tb, state=state):
                        nc.tensor.matmul(
                            state["yps"][1][:],
                            ot[0][:, tb * 128:(tb + 1) * 128],
                            wo_sb[:, 0, 512:1024],
                            start=True, stop=False,
                        )

                    def c2(tb=tb, state=state):
                        nc.tensor.matmul(
                            state["yps"][0][:],
                            ot[1][:, tb * 128:(tb + 1) * 128],
                            wo_sb[:, 1, 0:512],
                            start=False, stop=True,
                        )
                        nc.vector.tensor_copy(
                            state["ysb"][:, 0:512], state["yps"][0][:]
                        )

                    def c3(tb=tb, state=state):
                        nc.tensor.matmul(
                            state["yps"][1][:],
                            ot[1][:, tb * 128:(tb + 1) * 128],
                            wo_sb[:, 1, 512:1024],
                            start=False, stop=True,
                        )
                        nc.vector.tensor_copy(
                            state["ysb"][:, 512:1024], state["yps"][1][:]
                        )
                        nc.sync.dma_start(y_rows[tb], state["ysb"][:])

                    chunks += [c0, c1, c2, c3]
                return chunks

            def emit_E(qb):
                for ch in make_E_chunks(qb):
                    ch()

            # ---------------- pipelined emission ----------------
            # Phase 1: stages A/B per t-group, with CD(q0, pair0) k-iterations
            # interleaved so ScalarE ramps while the PE grinds projections.
            osA = psO.tile([65, 512], F32, tag="osA", name="osA0_0")
            osB = psO.tile([65, 512], F32, tag="osB", name="osB0_0")
            for tb in range(4):
                emit_A(tb)
                emit_B(tb)
                for kb in range(4 * tb, 4 * tb + 4):
                    p01 = emit_S_exp(0, 0, kb)
                    emit_D(0, 0, kb, p01, osA, osB)
            pending = [(0, 0, osA, osB)]

            # Phase 2: remaining groups; each group's first two S/exp pairs
            # are emitted before the previous group's epilogue so ACT stays fed
            # across the boundary. Norm steps and E chunks of the previous
            # q-block are slotted at fixed k-iterations so their PE pieces
            # never head-of-line-block on DVE results.
            groups = [(0, 1)] + [(qb, pi) for qb in range(1, NQB) for pi in range(2)]
            e_chunks = []
            norm_steps = []
            for qb, pi in groups:
                head = [emit_S_exp(qb, pi, kb) for kb in (0, 1)]
                pqb, ppi, posA, posB = pending.pop()
                emit_stage_out(pqb, ppi, posA, posB)
                if ppi == 1:
                    norm_steps = make_norm_steps(pqb)
                    e_chunks = make_E_chunks(pqb)
                osA = psO.tile([65, 512], F32, tag="osA", name=f"osA{qb}_{pi}")
                osB = psO.tile([65, 512], F32, tag="osB", name=f"osB{qb}_{pi}")
                for kb in (0, 1):
                    emit_D(qb, pi, kb, head[kb], osA, osB)
                for kb in range(2, NKB):
                    p01 = emit_S_exp(qb, pi, kb)
                    emit_D(qb, pi, kb, p01, osA, osB)
                    if norm_steps and kb in (2, 3, 5, 6, 8, 9):
                        norm_steps.pop(0)()
                    elif (not norm_steps and kb >= 9
                          and len(e_chunks) > (10 if (qb, pi) == groups[-1] else 0)):
                        e_chunks.pop(0)()
                        if kb >= 10 and len(e_chunks) > (
                                10 if (qb, pi) == groups[-1] else 0):
                            e_chunks.pop(0)()
                pending = [(qb, pi, osA, osB)]

            qb, pi, osA, osB = pending.pop()
            emit_stage_out(qb, pi, osA, osB)
            for s in make_norm_steps(qb):
                s()
                if e_chunks:
                    e_chunks.pop(0)()
            while e_chunks:
                e_chunks.pop(0)()
            emit_E(qb)

    nc.compile()
    return nc


def _get_nc(with_qkv_bias=True):
    key = ("nc", with_qkv_bias)
    if key not in _NC_CACHE:
        _NC_CACHE[key] = _build_nc(with_qkv_bias)
    return _NC_CACHE[key]


def _make_in_maps(x, boundary_score, W_qkv, b_qkv, W_out):
    x = np.asarray(x, np.float32)
    boundary_score = np.asarray(boundary_score, np.float32)
    W_qkv = np.asarray(W_qkv, np.float32)
    b_qkv = np.asarray(b_qkv, np.float32)
    W_out = np.asarray(W_out, np.float32)

    Wq, Wk, Wv = W_qkv[:, :D], W_qkv[:, D:2 * D], W_qkv[:, 2 * D:]
    bq, bk, bv = b_qkv[:D], b_qkv[D:2 * D], b_qkv[2 * D:]
    ones = np.ones((1, 512), np.float32)
    ones128 = np.ones((128, 64), np.float32)
    import ml_dtypes
    bf16 = ml_dtypes.bfloat16
    xts = [np.ascontiguousarray(x[b].T).astype(bf16) for b in range(x.shape[0])]

    in_maps = []
    for c in range(8):
        b, g = divmod(c, 4)
        lo, hi = 256 * g, 256 * (g + 1)
        wqk = np.ascontiguousarray(
            np.concatenate([Wq[:, lo:hi] * SCALE, Wk[:, lo:hi]], axis=1)
        ).astype(bf16)
        bqk = np.concatenate([bq[lo:hi] * SCALE, bk[lo:hi]])[None]
        wv = np.ascontiguousarray(Wv[:, lo:hi]).astype(bf16)
        bvv = np.ascontiguousarray(bv[lo:hi][None])
        wo = np.ascontiguousarray(W_out[lo:hi, :])
        bs = np.ascontiguousarray(
            (boundary_score[b] * BIAS_COEF).reshape(NKB, 128).T
        )
        in_maps.append(
            dict(
                xt=xts[b], wqk=wqk, bqk=np.ascontiguousarray(bqk),
                wv=wv, bv=bvv, wo=wo, bs=bs, ones=ones, ones128=ones128,
            )
        )
    return in_maps


def kernel(x, boundary_score, W_qkv, b_qkv, W_out, b_out):
    from concourse.bass_utils import run_bass_kernel_spmd

    x = np.asarray(x, np.float32)
    B = x.shape[0]
    in_maps = _make_in_maps(x, boundary_score, W_qkv, b_qkv, W_out)
    nc = _get_nc(with_qkv_bias=bool(np.any(np.asarray(b_qkv))))
    res = run_bass_kernel_spmd(nc, in_maps, list(range(8))).results
    out = np.zeros((B, T, D), np.float32)
    for c in range(8):
        out[c // 4] += np.asarray(res[c]["y"], np.float32)
    out += np.asarray(b_out, np.float32)
    return out



# revision 30
# speedup vs baseline: 1.1876x; 1.1867x over previous
"""BoundaryFluxAttention TRN2 kernel.

Distribution (8 cores): data-parallel over batch (B=2) x tensor-parallel over
heads (16 heads -> 4 groups of 4). Core c handles batch c//4, head group c%4.
Each core computes a partial output y_c = softmax-attention(its 4 heads) @ W_out
rows for those heads; the host sums the 4 partials per batch and adds b_out.

Per-core pipeline (T=2048, D=1024, 4 heads of hd=64), hand-pipelined emission:
  A:  QK^T projection qkt[db] [128, T] bf16 = (W slice)^T @ xT (fp32r matmuls);
      x arrives pre-transposed from the host. Scale hd^-0.5 folded into W_q/b_q.
  B:  V projection in natural [T, 256] layout -> vsb [128, kb, h, 65] bf16
      with a ones column at index 64 (denominator accumulates in the same
      matmul as O'^T).
  C:  S^T tiles [128k, 1024(2 heads)] = K_h^T.T @ Q_h^T, heads row-tiled
      (K=64 at partition offsets 0/64), bf16: ~113ns/matmul via PE row-group
      concurrency.
  exp: ScalarE, per-partition bias = boundary*0.1, bf16 out. This paces the
      CD loop (~1.1us per k-block) -> emission interleaves stage A/B and the
      previous group's epilogue so ACT never starves.
  D:  O'^T [65, 512] += V'_h.T @ P_h^T over k; row 64 = softmax denominator.
  norm: stage O' to SBUF (frees the PSUM accumulator fast), reciprocal of the
      denominator row, partition-broadcast via K=1 matmul, fused multiply.
      Odd heads shifted to partitions 64..127 via SBUF->SBUF DMA.
  E:  y = OT_pair @ W_out slice (fp32r), emitted per q-block as PE filler.
"""

import numpy as np

import concourse.bass as bass  # noqa: F401
import concourse.mybir as mybir
import concourse.tile as tile
from concourse import bacc

F32 = mybir.dt.float32
F32R = mybir.dt.float32r
BF16 = mybir.dt.bfloat16
EXP = mybir.ActivationFunctionType.Exp
FP16 = mybir.dt.float16

T = 2048
D = 1024
HPC = 4          # heads per core
HD = 64
NKB = T // 128   # 16 k/t blocks of 128
NQB = T // 512   # 4 q blocks of 512
NCH = D // 128   # 8 contraction chunks
SCALE = HD ** -0.5
BIAS_COEF = 0.1

_NC_CACHE = {}


def _ensure_patched_act_root():
    """Point walrus at an act_info.json with natural_log_exp_and_others
    listed first, so the kernel's Exp and Ln activations resolve to one
    table set (the default greedy order ping-pongs between exp_and_others
    and natural_log, costing a ~2.7us ACT_TABLE_LOAD per switch)."""
    import json
    import os
    import tempfile

    if os.environ.get("BASS_ACT_ROOT_JSON_PATH"):
        return
    try:
        from neuronxcc.driver.Job import Job
        from neuronxcc.driver.jobs.support.FindActInfo import findActInfoFile

        src_json = findActInfoFile(Job.getPackageDir(), "gen3")
    except Exception:
        return
    try:
        d = os.path.dirname(src_json)
        out = tempfile.mkdtemp(prefix="act_root_")
        for f in os.listdir(d):
            os.symlink(os.path.join(d, f), os.path.join(out, f))
        with open(src_json) as fh:
            ai = json.load(fh)
        sets = ai.get("act_func_sets", [])
        nle = [s for s in sets if s["name"] == "natural_log_exp_and_others"]
        if not nle:
            return
        ai["act_func_sets"] = nle + [
            s for s in sets if s["name"] != "natural_log_exp_and_others"
        ]
        os.remove(os.path.join(out, "act_info.json"))
        with open(os.path.join(out, "act_info.json"), "w") as fh:
            json.dump(ai, fh)
        os.environ["BASS_ACT_ROOT_JSON_PATH"] = os.path.join(out, "act_info.json")
    except Exception:
        pass



def _build_nc(with_qkv_bias=True):
    nc = bacc.Bacc("TRN2", target_bir_lowering=False)

    xt_d = nc.declare_dram_parameter("xt", [D, T], BF16, isOutput=False)
    wqk_d = nc.declare_dram_parameter("wqk", [D, 512], BF16, isOutput=False)
    bqk_d = nc.declare_dram_parameter("bqk", [1, 512], F32R, isOutput=False)
    wv_d = nc.declare_dram_parameter("wv", [D, 256], BF16, isOutput=False)
    bv_d = nc.declare_dram_parameter("bv", [1, 256], F32R, isOutput=False)
    wo_d = nc.declare_dram_parameter("wo", [256, D], F32R, isOutput=False)
    bs_d = nc.declare_dram_parameter("bs", [128, NKB], F32, isOutput=False)
    ones_d = nc.declare_dram_parameter("ones", [1, 512], F32R, isOutput=False)
    ones128_d = nc.declare_dram_parameter("ones128", [128, 64], F32R, isOutput=False)
    y_d = nc.declare_dram_parameter("y", [T, D], BF16, isOutput=True)

    with tile.TileContext(nc) as tc:
        with (
            tc.tile_pool(name="const", bufs=1) as constp,
            tc.tile_pool(name="wts", bufs=1) as wts,
            tc.tile_pool(name="big", bufs=1) as bigp,
            tc.tile_pool(name="pt", bufs=4) as ptp,
            tc.tile_pool(name="norm", bufs=1) as normp,
            tc.tile_pool(name="stg", bufs=2) as stgp,
            tc.tile_pool(name="ysb", bufs=3) as ypool,
            tc.tile_pool(name="psG", bufs=2, space="PSUM") as psG,
            tc.tile_pool(name="psS", bufs=2, space="PSUM") as psS,
            tc.tile_pool(name="psO", bufs=1, space="PSUM") as psO,
        ):
            # ---------------- constants / weights (scalar DMA queue) --------
            # x^T first, one big DMA per t-group on the sync queue: a single
            # DMA_DIRECT2D fans its packets across all 16 HW DMA engines, so
            # issuing 4 instead of 32 removes ~18us of SP-side issue
            # serialization and keeps arrival strictly t-group-ordered.
            xT = bigp.tile([128, NCH, T], BF16, tag="xT")
            xt_pct = xt_d.rearrange("(c p) t -> p c t", p=128)
            for tb in range(4):
                nc.sync.dma_start(
                    xT[:, :, tb * 512:(tb + 1) * 512],
                    xt_pct[:, :, tb * 512:(tb + 1) * 512],
                )

            wqk_sb = wts.tile([128, NCH, 512], BF16, tag="wqk")
            nc.scalar.dma_start(wqk_sb[:], wqk_d.rearrange("(c p) n -> p c n", p=128))
            bqk_sb = wts.tile([1, 512], F32R, tag="bqk")
            nc.scalar.dma_start(bqk_sb[:], bqk_d[:])
            wv_sb = wts.tile([128, NCH, 256], BF16, tag="wv")
            nc.scalar.dma_start(wv_sb[:], wv_d.rearrange("(c p) n -> p c n", p=128))
            bv_sb = wts.tile([1, 256], F32R, tag="bv")
            nc.scalar.dma_start(bv_sb[:], bv_d[:])

            ones = constp.tile([1, 512], F32R, tag="ones")
            nc.scalar.dma_start(ones[:], ones_d[:])
            ones128 = constp.tile([128, 64], F32R, tag="ones128")
            nc.scalar.dma_start(ones128[:], ones128_d[:])
            bs_sb = constp.tile([128, NKB], F32, tag="bs")
            nc.scalar.dma_start(bs_sb[:], bs_d[:])

            wo_sb = wts.tile([128, 2, D], F32R, tag="wo")
            nc.scalar.dma_start(wo_sb[:], wo_d.rearrange("(c p) n -> p c n", p=128))

            qkt = [
                bigp.tile([128, T], BF16, tag=f"qkt{db}", name=f"qkt{db}")
                for db in range(4)
            ]
            vsb = bigp.tile([128, NKB, HPC, 65], BF16, tag="vsb", name="vsb_v10")
            nc.gpsimd.memset(vsb[:], 1.0)
            ot = [
                bigp.tile([128, T], F32R, tag=f"ot{pi}", name=f"ot{pi}")
                for pi in range(2)
            ]
            y_rows = y_d.rearrange("(n p) d -> n p d", p=128)

            # ---------------- emission helpers ----------------
            def emit_A(tb):
                for db in range(4):
                    ps = psG.tile([128, 512], F32, tag="gp", name=f"qk{tb}_{db}")
                    for c in range(NCH):
                        nc.tensor.matmul(
                            ps[:],
                            wqk_sb[:, c, db * 128:(db + 1) * 128],
                            xT[:, c, tb * 512:(tb + 1) * 512],
                            start=(c == 0),
                            stop=(not with_qkv_bias and c == NCH - 1),
                        )
                    if with_qkv_bias:
                        nc.tensor.matmul(
                            ps[:],
                            bqk_sb[0:1, db * 128:(db + 1) * 128],
                            ones[0:1, :],
                            start=False,
                            stop=True,
                        )
                    nc.vector.tensor_copy(qkt[db][:, tb * 512:(tb + 1) * 512], ps[:])

            def emit_B(tb):
                for j in range(4):
                    kb = tb * 4 + j
                    ps = psG.tile([128, 256], F32, tag="gp", name=f"v{kb}")
                    for c in range(NCH):
                        nc.tensor.matmul(
                            ps[:],
                            xT[:, c, kb * 128:(kb + 1) * 128],
                            wv_sb[:, c, :],
                            start=(c == 0),
                            stop=(not with_qkv_bias and c == NCH - 1),
                        )
                    if with_qkv_bias:
                        nc.tensor.matmul(
                            ps[:], ones[0:1, 0:128], bv_sb[:], start=False, stop=True
                        )
                    nc.vector.tensor_copy(
                        vsb[:, kb, :, 0:64],
                        ps[:].rearrange("p (h c) -> p h c", h=HPC),
                    )

            def emit_S_exp(qb, pi, kb):
                qdb, kdb = pi, 2 + pi
                s01 = psS.tile([128, 1024], F32, tag="s01", name=f"s{qb}_{pi}_{kb}")
                nc.tensor.matmul(
                    s01[:, 0:512],
                    qkt[kdb][0:64, kb * 128:(kb + 1) * 128],
                    qkt[qdb][0:64, qb * 512:(qb + 1) * 512],
                )
                nc.tensor.matmul(
                    s01[:, 512:1024],
                    qkt[kdb][64:128, kb * 128:(kb + 1) * 128],
                    qkt[qdb][64:128, qb * 512:(qb + 1) * 512],
                )
                p01 = ptp.tile([128, 1024], BF16, tag="p01", name=f"p{qb}_{pi}_{kb}")
                nc.scalar.activation(p01[:], s01[:], EXP, bias=bs_sb[:, kb:kb + 1])
                return p01

            def emit_D(qb, pi, kb, p01, osA, osB):
                nc.tensor.matmul(
                    osA[:], vsb[:, kb, 2 * pi, :], p01[:, 0:512],
                    start=(kb == 0), stop=(kb == NKB - 1),
                )
                nc.tensor.matmul(
                    osB[:], vsb[:, kb, 2 * pi + 1, :], p01[:, 512:1024],
                    start=(kb == 0), stop=(kb == NKB - 1),
                )

            stgq_tiles = {}

            def emit_stage_out(qb, pi, osA, osB):
                # Stage O' out of PSUM immediately so the accumulator banks
                # free for the next group; normalization is deferred and
                # batched per q-block (emit_norm_qb).
                if pi == 0:
                    stgq_tiles[qb] = stgp.tile(
                        [65, 4, 512], F32R, tag="stgq", name=f"stgq{qb}"
                    )
                stgq = stgq_tiles[qb]
                nc.vector.tensor_copy(stgq[:, 2 * pi + 0, :], osA[:])
                nc.vector.tensor_copy(stgq[:, 2 * pi + 1, :], osB[:])

            def make_norm_steps(qb):
                # Normalization as schedulable steps so the PE never waits:
                #  step 0: 4 K=1 matmuls fan each head's raw denominator row
                #          into a 32-partition quadrant of one [128,512] PSUM
                #          tile, then ONE exact DVE reciprocal over all 128
                #          partitions (lane-parallel: ~3.2us, vs 12.9us on a
                #          single-partition row). ScalarE stays exp-only.
                #  steps 1..4: per-head K=1 rebroadcast of the f32r reciprocal
                #          + fused normalize-multiply (+ odd-head SBUF shift).
                stgq = stgq_tiles.pop(qb)
                cols = slice(qb * 512, (qb + 1) * 512)
                dt = normp.tile([128, 512], F32R, tag="dt", name=f"dt{qb}")
                rec = normp.tile([128, 512], F32R, tag="rec", name=f"rec{qb}")

                def s_dma(stgq=stgq, dt=dt):
                    nc.gpsimd.memset(dt[:].bitcast(F32), 1.0)
                    # Fan the 4 heads' denominator rows onto partitions
                    # 0/32/64/96 (tiny partition-moving SBUF DMAs) so ONE
                    # [128,512] DVE reciprocal covers all heads lane-parallel
                    # (~3.2us vs 12.9us on a single-partition row).
                    for j in range(4):
                        eng = nc.gpsimd if j % 2 == 0 else nc.sync
                        eng.dma_start(
                            dt[32 * j:32 * j + 1, :], stgq[64:65, j, :]
                        )

                def s_rec(dt=dt, rec=rec):
                    with nc.allow_low_precision(reason="f32r recip broadcast"):
                        nc.vector.reciprocal(
                            rec[0:97, :].opt(), dt[0:97, :].opt()
                        )

                steps = [s_dma, s_rec]
                for j in range(4):
                    def s_j(qb=qb, stgq=stgq, rec=rec, j=j):
                        pi, parity = divmod(j, 2)
                        bc2 = psG.tile(
                            [64, 512], F32, tag="gp", name=f"bc2_{qb}_{j}"
                        )
                        nc.tensor.matmul(
                            bc2[:], ones128[32 * j:32 * j + 1, 0:64],
                            rec[32 * j:32 * j + 1, :],
                            tile_position=(32 * j, 0),
                        )
                        if parity == 0:
                            nc.vector.tensor_mul(
                                ot[pi][0:64, cols], stgq[0:64, j, :], bc2[0:64, :]
                            )
                        else:
                            stag = normp.tile([64, 512], F32R, tag="stag")
                            nc.vector.tensor_mul(
                                stag[:], stgq[0:64, j, :], bc2[0:64, :]
                            )
                            nc.gpsimd.dma_start(ot[pi][64:128, cols], stag[:])
                    steps.append(s_j)
                return steps

            def make_E_chunks(qb):
                # Stage E for one q-block, sliced into 16 small closures so the
                # emission can interleave one chunk per k-iteration of the next
                # attention group (keeps PE dense without starving ScalarE).
                chunks = []
                for j in range(4):
                    tb = qb * 4 + j
                    state = {}

                    def c0(tb=tb, state=state):
                        state["ysb"] = ypool.tile(
                            [128, D], BF16, tag="ysb", name=f"ysb{tb}"
                        )
                        state["yps"] = [
                            psG.tile([128, 512], F32, tag="gp", name=f"yps{tb}_{nb}")
                            for nb in range(2)
                        ]
                        nc.tensor.matmul(
                            state["yps"][0][:],
                            ot[0][:, tb * 128:(tb + 1) * 128],
                            wo_sb[:, 0, 0:512],
                            start=True, stop=False,
                        )

                    def c1(tb=tb, state=state):
                        nc.tensor.matmul(
                            state["yps"][1][:],
                            ot[0][:, tb * 128:(tb + 1) * 128],
                            wo_sb[:, 0, 512:1024],
                            start=True, stop=False,
                        )

                    def c2(tb=tb, state=state):
                        nc.tensor.matmul(
                            state["yps"][0][:],
                            ot[1][:, tb * 128:(tb + 1) * 128],
                            wo_sb[:, 1, 0:512],
                            start=False, stop=True,
                        )
                        nc.vector.tensor_copy(
                            state["ysb"][:, 0:512], state["yps"][0][:]
                        )

                    def c3(tb=tb, state=state):
                        nc.tensor.matmul(
                            state["yps"][1][:],
                            ot[1][:, tb * 128:(tb + 1) * 128],
                            wo_sb[:, 1, 512:1024],
                            start=False, stop=True,
                        )
                        nc.vector.tensor_copy(
                            state["ysb"][:, 512:1024], state["yps"][1][:]
                        )
                        nc.sync.dma_start(y_rows[tb], state["ysb"][:])

                    chunks += [c0, c1, c2, c3]
                return chunks

            def emit_E(qb):
                for ch in make_E_chunks(qb):
                    ch()

            # ---------------- pipelined emission ----------------
            # Phase 1: stages A/B per t-group, with CD(q0, pair0) k-iterations
            # interleaved so ScalarE ramps while the PE grinds projections.
            osA = psO.tile([65, 512], F32, tag="osA", name="osA0_0")
            osB = psO.tile([65, 512], F32, tag="osB", name="osB0_0")
            for tb in range(4):
                emit_A(tb)
                emit_B(tb)
                for kb in range(4 * tb, 4 * tb + 4):
                    p01 = emit_S_exp(0, 0, kb)
                    emit_D(0, 0, kb, p01, osA, osB)
            pending = [(0, 0, osA, osB)]

            # Phase 2: remaining groups; each group's first two S/exp pairs
            # are emitted before the previous group's epilogue so ACT stays fed
            # across the boundary. Norm steps and E chunks of the previous
            # q-block are slotted at fixed k-iterations so their PE pieces
            # never head-of-line-block on DVE results.
            groups = [(0, 1)] + [(qb, pi) for qb in range(1, NQB) for pi in range(2)]
            e_chunks = []
            norm_steps = []
            for qb, pi in groups:
                head = [emit_S_exp(qb, pi, kb) for kb in (0, 1)]
                pqb, ppi, posA, posB = pending.pop()
                emit_stage_out(pqb, ppi, posA, posB)
                if ppi == 1:
                    norm_steps = make_norm_steps(pqb)
                    e_chunks = make_E_chunks(pqb)
                osA = psO.tile([65, 512], F32, tag="osA", name=f"osA{qb}_{pi}")
                osB = psO.tile([65, 512], F32, tag="osB", name=f"osB{qb}_{pi}")
                for kb in (0, 1):
                    emit_D(qb, pi, kb, head[kb], osA, osB)
                for kb in range(2, NKB):
                    p01 = emit_S_exp(qb, pi, kb)
                    emit_D(qb, pi, kb, p01, osA, osB)
                    if norm_steps and kb in (2, 3, 5, 6, 8, 9):
                        norm_steps.pop(0)()
                    elif (not norm_steps and kb >= 10
                          and len(e_chunks) > (6 if (qb, pi) == groups[-1] else 0)):
                        e_chunks.pop(0)()
                        if kb >= 11 and len(e_chunks) > (
                                6 if (qb, pi) == groups[-1] else 0):
                            e_chunks.pop(0)()
                pending = [(qb, pi, osA, osB)]

            qb, pi, osA, osB = pending.pop()
            emit_stage_out(qb, pi, osA, osB)
            for s in make_norm_steps(qb):
                s()
                if e_chunks:
                    e_chunks.pop(0)()
            while e_chunks:
                e_chunks.pop(0)()
            emit_E(qb)

    nc.compile()
    return nc


def _get_nc(with_qkv_bias=True):
    key = ("nc", with_qkv_bias)
    if key not in _NC_CACHE:
        _NC_CACHE[key] = _build_nc(with_qkv_bias)
    return _NC_CACHE[key]


def _make_in_maps(x, boundary_score, W_qkv, b_qkv, W_out):
    x = np.asarray(x, np.float32)
    boundary_score = np.asarray(boundary_score, np.float32)
    W_qkv = np.asarray(W_qkv, np.float32)
    b_qkv = np.asarray(b_qkv, np.float32)
    W_out = np.asarray(W_out, np.float32)

    Wq, Wk, Wv = W_qkv[:, :D], W_qkv[:, D:2 * D], W_qkv[:, 2 * D:]
    bq, bk, bv = b_qkv[:D], b_qkv[D:2 * D], b_qkv[2 * D:]
    ones = np.ones((1, 512), np.float32)
    ones128 = np.ones((128, 64), np.float32)
    import ml_dtypes
    bf16 = ml_dtypes.bfloat16
    xts = [np.ascontiguousarray(x[b].T).astype(bf16) for b in range(x.shape[0])]

    in_maps = []
    for c in range(8):
        b, g = divmod(c, 4)
        lo, hi = 256 * g, 256 * (g + 1)
        wqk = np.ascontiguousarray(
            np.concatenate([Wq[:, lo:hi] * SCALE, Wk[:, lo:hi]], axis=1)
        ).astype(bf16)
        bqk = np.concatenate([bq[lo:hi] * SCALE, bk[lo:hi]])[None]
        wv = np.ascontiguousarray(Wv[:, lo:hi]).astype(bf16)
        bvv = np.ascontiguousarray(bv[lo:hi][None])
        wo = np.ascontiguousarray(W_out[lo:hi, :])
        bs = np.ascontiguousarray(
            (boundary_score[b] * BIAS_COEF).reshape(NKB, 128).T
        )
        in_maps.append(
            dict(
                xt=xts[b], wqk=wqk, bqk=np.ascontiguousarray(bqk),
                wv=wv, bv=bvv, wo=wo, bs=bs, ones=ones, ones128=ones128,
            )
        )
    return in_maps


def kernel(x, boundary_score, W_qkv, b_qkv, W_out, b_out):
    from concourse.bass_utils import run_bass_kernel_spmd

    x = np.asarray(x, np.float32)
    B = x.shape[0]
    in_maps = _make_in_maps(x, boundary_score, W_qkv, b_qkv, W_out)
    nc = _get_nc(with_qkv_bias=bool(np.any(np.asarray(b_qkv))))
    res = run_bass_kernel_spmd(nc, in_maps, list(range(8))).results
    out = np.zeros((B, T, D), np.float32)
    for c in range(8):
        out[c // 4] += np.asarray(res[c]["y"], np.float32)
    out += np.asarray(b_out, np.float32)
    return out



# revision 32
# speedup vs baseline: 1.1895x; 1.0017x over previous
"""BoundaryFluxAttention TRN2 kernel.

Distribution (8 cores): data-parallel over batch (B=2) x tensor-parallel over
heads (16 heads -> 4 groups of 4). Core c handles batch c//4, head group c%4.
Each core computes a partial output y_c = softmax-attention(its 4 heads) @ W_out
rows for those heads; the host sums the 4 partials per batch and adds b_out.

Per-core pipeline (T=2048, D=1024, 4 heads of hd=64), hand-pipelined emission:
  A:  QK^T projection qkt[db] [128, T] bf16 = (W slice)^T @ xT (fp32r matmuls);
      x arrives pre-transposed from the host. Scale hd^-0.5 folded into W_q/b_q.
  B:  V projection in natural [T, 256] layout -> vsb [128, kb, h, 65] bf16
      with a ones column at index 64 (denominator accumulates in the same
      matmul as O'^T).
  C:  S^T tiles [128k, 1024(2 heads)] = K_h^T.T @ Q_h^T, heads row-tiled
      (K=64 at partition offsets 0/64), bf16: ~113ns/matmul via PE row-group
      concurrency.
  exp: ScalarE, per-partition bias = boundary*0.1, bf16 out. This paces the
      CD loop (~1.1us per k-block) -> emission interleaves stage A/B and the
      previous group's epilogue so ACT never starves.
  D:  O'^T [65, 512] += V'_h.T @ P_h^T over k; row 64 = softmax denominator.
  norm: stage O' to SBUF (frees the PSUM accumulator fast), reciprocal of the
      denominator row, partition-broadcast via K=1 matmul, fused multiply.
      Odd heads shifted to partitions 64..127 via SBUF->SBUF DMA.
  E:  y = OT_pair @ W_out slice (fp32r), emitted per q-block as PE filler.
"""

import numpy as np

import concourse.bass as bass  # noqa: F401
import concourse.mybir as mybir
import concourse.tile as tile
from concourse import bacc

F32 = mybir.dt.float32
F32R = mybir.dt.float32r
BF16 = mybir.dt.bfloat16
EXP = mybir.ActivationFunctionType.Exp
FP16 = mybir.dt.float16

T = 2048
D = 1024
HPC = 4          # heads per core
HD = 64
NKB = T // 128   # 16 k/t blocks of 128
NQB = T // 512   # 4 q blocks of 512
NCH = D // 128   # 8 contraction chunks
SCALE = HD ** -0.5
BIAS_COEF = 0.1

_NC_CACHE = {}


def _ensure_patched_act_root():
    """Point walrus at an act_info.json with natural_log_exp_and_others
    listed first, so the kernel's Exp and Ln activations resolve to one
    table set (the default greedy order ping-pongs between exp_and_others
    and natural_log, costing a ~2.7us ACT_TABLE_LOAD per switch)."""
    import json
    import os
    import tempfile

    if os.environ.get("BASS_ACT_ROOT_JSON_PATH"):
        return
    try:
        from neuronxcc.driver.Job import Job
        from neuronxcc.driver.jobs.support.FindActInfo import findActInfoFile

        src_json = findActInfoFile(Job.getPackageDir(), "gen3")
    except Exception:
        return
    try:
        d = os.path.dirname(src_json)
        out = tempfile.mkdtemp(prefix="act_root_")
        for f in os.listdir(d):
            os.symlink(os.path.join(d, f), os.path.join(out, f))
        with open(src_json) as fh:
            ai = json.load(fh)
        sets = ai.get("act_func_sets", [])
        nle = [s for s in sets if s["name"] == "natural_log_exp_and_others"]
        if not nle:
            return
        ai["act_func_sets"] = nle + [
            s for s in sets if s["name"] != "natural_log_exp_and_others"
        ]
        os.remove(os.path.join(out, "act_info.json"))
        with open(os.path.join(out, "act_info.json"), "w") as fh:
            json.dump(ai, fh)
        os.environ["BASS_ACT_ROOT_JSON_PATH"] = os.path.join(out, "act_info.json")
    except Exception:
        pass



def _build_nc(with_qkv_bias=True):
    nc = bacc.Bacc("TRN2", target_bir_lowering=False)

    xt_d = nc.declare_dram_parameter("xt", [D, T], BF16, isOutput=False)
    wqk_d = nc.declare_dram_parameter("wqk", [D, 512], BF16, isOutput=False)
    bqk_d = nc.declare_dram_parameter("bqk", [1, 512], F32R, isOutput=False)
    wv_d = nc.declare_dram_parameter("wv", [D, 256], BF16, isOutput=False)
    bv_d = nc.declare_dram_parameter("bv", [1, 256], F32R, isOutput=False)
    wo_d = nc.declare_dram_parameter("wo", [256, D], F32R, isOutput=False)
    bs_d = nc.declare_dram_parameter("bs", [128, NKB], F32, isOutput=False)
    ones_d = nc.declare_dram_parameter("ones", [1, 512], F32R, isOutput=False)
    ones128_d = nc.declare_dram_parameter("ones128", [128, 64], F32R, isOutput=False)
    y_d = nc.declare_dram_parameter("y", [T, D], BF16, isOutput=True)

    with tile.TileContext(nc) as tc:
        with (
            tc.tile_pool(name="const", bufs=1) as constp,
            tc.tile_pool(name="wts", bufs=1) as wts,
            tc.tile_pool(name="big", bufs=1) as bigp,
            tc.tile_pool(name="pt", bufs=4) as ptp,
            tc.tile_pool(name="norm", bufs=1) as normp,
            tc.tile_pool(name="stg", bufs=2) as stgp,
            tc.tile_pool(name="ysb", bufs=3) as ypool,
            tc.tile_pool(name="psG", bufs=2, space="PSUM") as psG,
            tc.tile_pool(name="psS", bufs=2, space="PSUM") as psS,
            tc.tile_pool(name="psO", bufs=1, space="PSUM") as psO,
        ):
            # ---------------- constants / weights (scalar DMA queue) --------
            # x^T first, one big DMA per t-group on the sync queue: a single
            # DMA_DIRECT2D fans its packets across all 16 HW DMA engines, so
            # issuing 4 instead of 32 removes ~18us of SP-side issue
            # serialization and keeps arrival strictly t-group-ordered.
            xT = bigp.tile([128, NCH, T], BF16, tag="xT")
            xt_pct = xt_d.rearrange("(c p) t -> p c t", p=128)
            for tb in range(4):
                nc.sync.dma_start(
                    xT[:, :, tb * 512:(tb + 1) * 512],
                    xt_pct[:, :, tb * 512:(tb + 1) * 512],
                )

            wqk_sb = wts.tile([128, NCH, 512], BF16, tag="wqk")
            nc.scalar.dma_start(wqk_sb[:], wqk_d.rearrange("(c p) n -> p c n", p=128))
            bqk_sb = wts.tile([1, 512], F32R, tag="bqk")
            nc.scalar.dma_start(bqk_sb[:], bqk_d[:])
            wv_sb = wts.tile([128, NCH, 256], BF16, tag="wv")
            nc.scalar.dma_start(wv_sb[:], wv_d.rearrange("(c p) n -> p c n", p=128))
            bv_sb = wts.tile([1, 256], F32R, tag="bv")
            nc.scalar.dma_start(bv_sb[:], bv_d[:])

            ones = constp.tile([1, 512], F32R, tag="ones")
            nc.scalar.dma_start(ones[:], ones_d[:])
            ones128 = constp.tile([128, 64], F32R, tag="ones128")
            nc.scalar.dma_start(ones128[:], ones128_d[:])
            bs_sb = constp.tile([128, NKB], F32, tag="bs")
            nc.scalar.dma_start(bs_sb[:], bs_d[:])

            wo_sb = wts.tile([128, 2, D], F32R, tag="wo")
            nc.scalar.dma_start(wo_sb[:], wo_d.rearrange("(c p) n -> p c n", p=128))

            qkt = [
                bigp.tile([128, T], BF16, tag=f"qkt{db}", name=f"qkt{db}")
                for db in range(4)
            ]
            vsb = bigp.tile([128, NKB, HPC, 65], BF16, tag="vsb", name="vsb_v10")
            nc.gpsimd.memset(vsb[:], 1.0)
            ot = [
                bigp.tile([128, T], F32R, tag=f"ot{pi}", name=f"ot{pi}")
                for pi in range(2)
            ]
            y_rows = y_d.rearrange("(n p) d -> n p d", p=128)

            # ---------------- emission helpers ----------------
            def emit_A(tb):
                for db in range(4):
                    ps = psG.tile([128, 512], F32, tag="gp", name=f"qk{tb}_{db}")
                    for c in range(NCH):
                        nc.tensor.matmul(
                            ps[:],
                            wqk_sb[:, c, db * 128:(db + 1) * 128],
                            xT[:, c, tb * 512:(tb + 1) * 512],
                            start=(c == 0),
                            stop=(not with_qkv_bias and c == NCH - 1),
                        )
                    if with_qkv_bias:
                        nc.tensor.matmul(
                            ps[:],
                            bqk_sb[0:1, db * 128:(db + 1) * 128],
                            ones[0:1, :],
                            start=False,
                            stop=True,
                        )
                    nc.vector.tensor_copy(qkt[db][:, tb * 512:(tb + 1) * 512], ps[:])

            def emit_B(tb):
                for j in range(4):
                    kb = tb * 4 + j
                    ps = psG.tile([128, 256], F32, tag="gp", name=f"v{kb}")
                    for c in range(NCH):
                        nc.tensor.matmul(
                            ps[:],
                            xT[:, c, kb * 128:(kb + 1) * 128],
                            wv_sb[:, c, :],
                            start=(c == 0),
                            stop=(not with_qkv_bias and c == NCH - 1),
                        )
                    if with_qkv_bias:
                        nc.tensor.matmul(
                            ps[:], ones[0:1, 0:128], bv_sb[:], start=False, stop=True
                        )
                    nc.vector.tensor_copy(
                        vsb[:, kb, :, 0:64],
                        ps[:].rearrange("p (h c) -> p h c", h=HPC),
                    )

            def emit_S_exp(qb, pi, kb):
                qdb, kdb = pi, 2 + pi
                s01 = psS.tile([128, 1024], F32, tag="s01", name=f"s{qb}_{pi}_{kb}")
                nc.tensor.matmul(
                    s01[:, 0:512],
                    qkt[kdb][0:64, kb * 128:(kb + 1) * 128],
                    qkt[qdb][0:64, qb * 512:(qb + 1) * 512],
                )
                nc.tensor.matmul(
                    s01[:, 512:1024],
                    qkt[kdb][64:128, kb * 128:(kb + 1) * 128],
                    qkt[qdb][64:128, qb * 512:(qb + 1) * 512],
                )
                p01 = ptp.tile([128, 1024], BF16, tag="p01", name=f"p{qb}_{pi}_{kb}")
                nc.scalar.activation(p01[:], s01[:], EXP, bias=bs_sb[:, kb:kb + 1])
                return p01

            def emit_D(qb, pi, kb, p01, osA, osB):
                nc.tensor.matmul(
                    osA[:], vsb[:, kb, 2 * pi, :], p01[:, 0:512],
                    start=(kb == 0), stop=(kb == NKB - 1),
                )
                nc.tensor.matmul(
                    osB[:], vsb[:, kb, 2 * pi + 1, :], p01[:, 512:1024],
                    start=(kb == 0), stop=(kb == NKB - 1),
                )

            stgq_tiles = {}

            def emit_stage_out(qb, pi, osA, osB):
                # Stage O' out of PSUM immediately so the accumulator banks
                # free for the next group; normalization is deferred and
                # batched per q-block (emit_norm_qb).
                if pi == 0:
                    stgq_tiles[qb] = stgp.tile(
                        [65, 4, 512], F32R, tag="stgq", name=f"stgq{qb}"
                    )
                stgq = stgq_tiles[qb]
                nc.vector.tensor_copy(stgq[:, 2 * pi + 0, :], osA[:])
                nc.vector.tensor_copy(stgq[:, 2 * pi + 1, :], osB[:])

            def make_norm_steps(qb):
                # Normalization as schedulable steps so the PE never waits:
                #  step 0: 4 K=1 matmuls fan each head's raw denominator row
                #          into a 32-partition quadrant of one [128,512] PSUM
                #          tile, then ONE exact DVE reciprocal over all 128
                #          partitions (lane-parallel: ~3.2us, vs 12.9us on a
                #          single-partition row). ScalarE stays exp-only.
                #  steps 1..4: per-head K=1 rebroadcast of the f32r reciprocal
                #          + fused normalize-multiply (+ odd-head SBUF shift).
                stgq = stgq_tiles.pop(qb)
                cols = slice(qb * 512, (qb + 1) * 512)
                dt = normp.tile([128, 512], F32R, tag="dt", name=f"dt{qb}")
                rec = normp.tile([128, 512], F32R, tag="rec", name=f"rec{qb}")

                def s_dma(stgq=stgq, dt=dt):
                    nc.gpsimd.memset(dt[:].bitcast(F32), 1.0)
                    # Fan the 4 heads' denominator rows onto partitions
                    # 0/32/64/96 (tiny partition-moving SBUF DMAs) so ONE
                    # [128,512] DVE reciprocal covers all heads lane-parallel
                    # (~3.2us vs 12.9us on a single-partition row).
                    for j in range(4):
                        eng = nc.gpsimd if j % 2 == 0 else nc.sync
                        eng.dma_start(
                            dt[32 * j:32 * j + 1, :], stgq[64:65, j, :]
                        )

                def s_rec(dt=dt, rec=rec):
                    with nc.allow_low_precision(reason="f32r recip broadcast"):
                        nc.vector.reciprocal(
                            rec[0:97, :].opt(), dt[0:97, :].opt()
                        )

                steps = [s_dma, s_rec]
                for j in range(4):
                    def s_j(qb=qb, stgq=stgq, rec=rec, j=j):
                        pi, parity = divmod(j, 2)
                        bc2 = psG.tile(
                            [64, 512], F32, tag="gp", name=f"bc2_{qb}_{j}"
                        )
                        nc.tensor.matmul(
                            bc2[:], ones128[32 * j:32 * j + 1, 0:64],
                            rec[32 * j:32 * j + 1, :],
                            tile_position=(32 * j, 0),
                        )
                        if parity == 0:
                            nc.vector.tensor_mul(
                                ot[pi][0:64, cols], stgq[0:64, j, :], bc2[0:64, :]
                            )
                        else:
                            stag = normp.tile([64, 512], F32R, tag="stag")
                            nc.vector.tensor_mul(
                                stag[:], stgq[0:64, j, :], bc2[0:64, :]
                            )
                            nc.gpsimd.dma_start(ot[pi][64:128, cols], stag[:])
                    steps.append(s_j)
                return steps

            def make_E_chunks(qb):
                # Stage E for one q-block, sliced into 16 small closures so the
                # emission can interleave one chunk per k-iteration of the next
                # attention group (keeps PE dense without starving ScalarE).
                chunks = []
                for j in range(4):
                    tb = qb * 4 + j
                    state = {}

                    def c0(tb=tb, state=state):
                        state["ysb"] = ypool.tile(
                            [128, D], BF16, tag="ysb", name=f"ysb{tb}"
                        )
                        state["yps"] = [
                            psG.tile([128, 512], F32, tag="gp", name=f"yps{tb}_{nb}")
                            for nb in range(2)
                        ]
                        nc.tensor.matmul(
                            state["yps"][0][:],
                            ot[0][:, tb * 128:(tb + 1) * 128],
                            wo_sb[:, 0, 0:512],
                            start=True, stop=False,
                        )

                    def c1(tb=tb, state=state):
                        nc.tensor.matmul(
                            state["yps"][1][:],
                            ot[0][:, tb * 128:(tb + 1) * 128],
                            wo_sb[:, 0, 512:1024],
                            start=True, stop=False,
                        )

                    def c2(tb=tb, state=state):
                        nc.tensor.matmul(
                            state["yps"][0][:],
                            ot[1][:, tb * 128:(tb + 1) * 128],
                            wo_sb[:, 1, 0:512],
                            start=False, stop=True,
                        )
                        nc.vector.tensor_copy(
                            state["ysb"][:, 0:512], state["yps"][0][:]
                        )

                    def c3(tb=tb, state=state):
                        nc.tensor.matmul(
                            state["yps"][1][:],
                            ot[1][:, tb * 128:(tb + 1) * 128],
                            wo_sb[:, 1, 512:1024],
                            start=False, stop=True,
                        )
                        nc.vector.tensor_copy(
                            state["ysb"][:, 512:1024], state["yps"][1][:]
                        )
                        nc.sync.dma_start(y_rows[tb], state["ysb"][:])

                    chunks += [c0, c1, c2, c3]
                return chunks

            def emit_E(qb):
                for ch in make_E_chunks(qb):
                    ch()

            # ---------------- pipelined emission ----------------
            # Phase 1: stages A/B per t-group, with CD(q0, pair0) k-iterations
            # interleaved so ScalarE ramps while the PE grinds projections.
            osA = psO.tile([65, 512], F32, tag="osA", name="osA0_0")
            osB = psO.tile([65, 512], F32, tag="osB", name="osB0_0")
            for tb in range(4):
                emit_A(tb)
                emit_B(tb)
                for kb in range(4 * tb, 4 * tb + 4):
                    p01 = emit_S_exp(0, 0, kb)
                    emit_D(0, 0, kb, p01, osA, osB)
            pending = [(0, 0, osA, osB)]

            # Phase 2: remaining groups; each group's first two S/exp pairs
            # are emitted before the previous group's epilogue so ACT stays fed
            # across the boundary. Norm steps and E chunks of the previous
            # q-block are slotted at fixed k-iterations so their PE pieces
            # never head-of-line-block on DVE results.
            groups = [(0, 1)] + [(qb, pi) for qb in range(1, NQB) for pi in range(2)]
            e_chunks = []
            norm_steps = []
            for qb, pi in groups:
                head = [emit_S_exp(qb, pi, kb) for kb in (0, 1)]
                pqb, ppi, posA, posB = pending.pop()
                emit_stage_out(pqb, ppi, posA, posB)
                if ppi == 1:
                    norm_steps = make_norm_steps(pqb)
                    e_chunks = make_E_chunks(pqb)
                osA = psO.tile([65, 512], F32, tag="osA", name=f"osA{qb}_{pi}")
                osB = psO.tile([65, 512], F32, tag="osB", name=f"osB{qb}_{pi}")
                for kb in (0, 1):
                    emit_D(qb, pi, kb, head[kb], osA, osB)
                for kb in range(2, NKB):
                    p01 = emit_S_exp(qb, pi, kb)
                    emit_D(qb, pi, kb, p01, osA, osB)
                    if norm_steps and kb in (2, 3, 5, 6, 8, 9):
                        norm_steps.pop(0)()
                    elif (not norm_steps and kb >= 10
                          and len(e_chunks) > (6 if (qb, pi) == groups[-1] else 0)):
                        e_chunks.pop(0)()
                        if kb >= 11 and len(e_chunks) > (
                                6 if (qb, pi) == groups[-1] else 0):
                            e_chunks.pop(0)()
                pending = [(qb, pi, osA, osB)]

            qb, pi, osA, osB = pending.pop()
            emit_stage_out(qb, pi, osA, osB)
            for s in make_norm_steps(qb):
                s()
                if e_chunks:
                    e_chunks.pop(0)()
            while e_chunks:
                e_chunks.pop(0)()
            emit_E(qb)

    nc.compile()
    return nc


def _get_nc(with_qkv_bias=True):
    key = ("nc", with_qkv_bias)
    if key not in _NC_CACHE:
        _NC_CACHE[key] = _build_nc(with_qkv_bias)
    return _NC_CACHE[key]


def _make_in_maps(x, boundary_score, W_qkv, b_qkv, W_out):
    x = np.asarray(x, np.float32)
    boundary_score = np.asarray(boundary_score, np.float32)
    W_qkv = np.asarray(W_qkv, np.float32)
    b_qkv = np.asarray(b_qkv, np.float32)
    W_out = np.asarray(W_out, np.float32)

    Wq, Wk, Wv = W_qkv[:, :D], W_qkv[:, D:2 * D], W_qkv[:, 2 * D:]
    bq, bk, bv = b_qkv[:D], b_qkv[D:2 * D], b_qkv[2 * D:]
    ones = np.ones((1, 512), np.float32)
    ones128 = np.ones((128, 64), np.float32)
    import ml_dtypes
    bf16 = ml_dtypes.bfloat16
    xts = [np.ascontiguousarray(x[b].T).astype(bf16) for b in range(x.shape[0])]

    in_maps = []
    for c in range(8):
        b, g = divmod(c, 4)
        lo, hi = 256 * g, 256 * (g + 1)
        wqk = np.ascontiguousarray(
            np.concatenate([Wq[:, lo:hi] * SCALE, Wk[:, lo:hi]], axis=1)
        ).astype(bf16)
        bqk = np.concatenate([bq[lo:hi] * SCALE, bk[lo:hi]])[None]
        wv = np.ascontiguousarray(Wv[:, lo:hi]).astype(bf16)
        bvv = np.ascontiguousarray(bv[lo:hi][None])
        wo = np.ascontiguousarray(W_out[lo:hi, :])
        bs = np.ascontiguousarray(
            (boundary_score[b] * BIAS_COEF).reshape(NKB, 128).T
        )
        in_maps.append(
            dict(
                xt=xts[b], wqk=wqk, bqk=np.ascontiguousarray(bqk),
                wv=wv, bv=bvv, wo=wo, bs=bs, ones=ones, ones128=ones128,
            )
        )
    return in_maps


def kernel(x, boundary_score, W_qkv, b_qkv, W_out, b_out):
    from concourse.bass_utils import run_bass_kernel_spmd

    x = np.asarray(x, np.float32)
    B = x.shape[0]
    in_maps = _make_in_maps(x, boundary_score, W_qkv, b_qkv, W_out)
    nc = _get_nc(with_qkv_bias=bool(np.any(np.asarray(b_qkv))))
    res = run_bass_kernel_spmd(nc, in_maps, list(range(8))).results
    out = np.zeros((B, T, D), np.float32)
    for c in range(8):
        out[c // 4] += np.asarray(res[c]["y"], np.float32)
    out += np.asarray(b_out, np.float32)
    return out



# revision 33
# speedup vs baseline: 1.1969x; 1.0062x over previous
"""BoundaryFluxAttention TRN2 kernel.

Distribution (8 cores): data-parallel over batch (B=2) x tensor-parallel over
heads (16 heads -> 4 groups of 4). Core c handles batch c//4, head group c%4.
Each core computes a partial output y_c = softmax-attention(its 4 heads) @ W_out
rows for those heads; the host sums the 4 partials per batch and adds b_out.

Per-core pipeline (T=2048, D=1024, 4 heads of hd=64), hand-pipelined emission:
  A:  QK^T projection qkt[db] [128, T] bf16 = (W slice)^T @ xT (fp32r matmuls);
      x arrives pre-transposed from the host. Scale hd^-0.5 folded into W_q/b_q.
  B:  V projection in natural [T, 256] layout -> vsb [128, kb, h, 65] bf16
      with a ones column at index 64 (denominator accumulates in the same
      matmul as O'^T).
  C:  S^T tiles [128k, 1024(2 heads)] = K_h^T.T @ Q_h^T, heads row-tiled
      (K=64 at partition offsets 0/64), bf16: ~113ns/matmul via PE row-group
      concurrency.
  exp: ScalarE, per-partition bias = boundary*0.1, bf16 out. This paces the
      CD loop (~1.1us per k-block) -> emission interleaves stage A/B and the
      previous group's epilogue so ACT never starves.
  D:  O'^T [65, 512] += V'_h.T @ P_h^T over k; row 64 = softmax denominator.
  norm: stage O' to SBUF (frees the PSUM accumulator fast), reciprocal of the
      denominator row, partition-broadcast via K=1 matmul, fused multiply.
      Odd heads shifted to partitions 64..127 via SBUF->SBUF DMA.
  E:  y = OT_pair @ W_out slice (fp32r), emitted per q-block as PE filler.
"""

import numpy as np

import concourse.bass as bass  # noqa: F401
import concourse.mybir as mybir
import concourse.tile as tile
from concourse import bacc

F32 = mybir.dt.float32
F32R = mybir.dt.float32r
BF16 = mybir.dt.bfloat16
EXP = mybir.ActivationFunctionType.Exp
FP16 = mybir.dt.float16

T = 2048
D = 1024
HPC = 4          # heads per core
HD = 64
NKB = T // 128   # 16 k/t blocks of 128
NQB = T // 512   # 4 q blocks of 512
NCH = D // 128   # 8 contraction chunks
SCALE = HD ** -0.5
BIAS_COEF = 0.1

_NC_CACHE = {}


def _ensure_patched_act_root():
    """Point walrus at an act_info.json with natural_log_exp_and_others
    listed first, so the kernel's Exp and Ln activations resolve to one
    table set (the default greedy order ping-pongs between exp_and_others
    and natural_log, costing a ~2.7us ACT_TABLE_LOAD per switch)."""
    import json
    import os
    import tempfile

    if os.environ.get("BASS_ACT_ROOT_JSON_PATH"):
        return
    try:
        from neuronxcc.driver.Job import Job
        from neuronxcc.driver.jobs.support.FindActInfo import findActInfoFile

        src_json = findActInfoFile(Job.getPackageDir(), "gen3")
    except Exception:
        return
    try:
        d = os.path.dirname(src_json)
        out = tempfile.mkdtemp(prefix="act_root_")
        for f in os.listdir(d):
            os.symlink(os.path.join(d, f), os.path.join(out, f))
        with open(src_json) as fh:
            ai = json.load(fh)
        sets = ai.get("act_func_sets", [])
        nle = [s for s in sets if s["name"] == "natural_log_exp_and_others"]
        if not nle:
            return
        ai["act_func_sets"] = nle + [
            s for s in sets if s["name"] != "natural_log_exp_and_others"
        ]
        os.remove(os.path.join(out, "act_info.json"))
        with open(os.path.join(out, "act_info.json"), "w") as fh:
            json.dump(ai, fh)
        os.environ["BASS_ACT_ROOT_JSON_PATH"] = os.path.join(out, "act_info.json")
    except Exception:
        pass



def _build_nc(with_qkv_bias=True):
    nc = bacc.Bacc("TRN2", target_bir_lowering=False)

    xt_d = nc.declare_dram_parameter("xt", [D, T], BF16, isOutput=False)
    wqk_d = nc.declare_dram_parameter("wqk", [D, 512], BF16, isOutput=False)
    bqk_d = nc.declare_dram_parameter("bqk", [1, 512], F32R, isOutput=False)
    wv_d = nc.declare_dram_parameter("wv", [D, 256], BF16, isOutput=False)
    bv_d = nc.declare_dram_parameter("bv", [1, 256], F32R, isOutput=False)
    wo_d = nc.declare_dram_parameter("wo", [256, D], F32R, isOutput=False)
    bs_d = nc.declare_dram_parameter("bs", [128, NKB], F32, isOutput=False)
    ones_d = nc.declare_dram_parameter("ones", [1, 512], F32R, isOutput=False)
    ones128_d = nc.declare_dram_parameter("ones128", [128, 64], F32R, isOutput=False)
    y_d = nc.declare_dram_parameter("y", [T, D], BF16, isOutput=True)

    with tile.TileContext(nc) as tc:
        with (
            tc.tile_pool(name="const", bufs=1) as constp,
            tc.tile_pool(name="wts", bufs=1) as wts,
            tc.tile_pool(name="big", bufs=1) as bigp,
            tc.tile_pool(name="pt", bufs=4) as ptp,
            tc.tile_pool(name="norm", bufs=1) as normp,
            tc.tile_pool(name="stg", bufs=2) as stgp,
            tc.tile_pool(name="ysb", bufs=3) as ypool,
            tc.tile_pool(name="psG", bufs=2, space="PSUM") as psG,
            tc.tile_pool(name="psS", bufs=2, space="PSUM") as psS,
            tc.tile_pool(name="psO", bufs=1, space="PSUM") as psO,
        ):
            # ---------------- constants / weights (scalar DMA queue) --------
            # x^T first, one big DMA per t-group on the sync queue: a single
            # DMA_DIRECT2D fans its packets across all 16 HW DMA engines, so
            # issuing 4 instead of 32 removes ~18us of SP-side issue
            # serialization and keeps arrival strictly t-group-ordered.
            xT = bigp.tile([128, NCH, T], BF16, tag="xT")
            xt_pct = xt_d.rearrange("(c p) t -> p c t", p=128)
            for c0_, c1_ in ((0, 4), (4, 8)):
                nc.sync.dma_start(
                    xT[:, c0_:c1_, 0:512], xt_pct[:, c0_:c1_, 0:512]
                )
            for tb in range(1, 4):
                nc.sync.dma_start(
                    xT[:, :, tb * 512:(tb + 1) * 512],
                    xt_pct[:, :, tb * 512:(tb + 1) * 512],
                )

            wqk_sb = wts.tile([128, NCH, 512], BF16, tag="wqk")
            wqk_pcn = wqk_d.rearrange("(c p) n -> p c n", p=128)
            nc.scalar.dma_start(wqk_sb[:, :, 0:128], wqk_pcn[:, :, 0:128])
            nc.scalar.dma_start(wqk_sb[:, :, 128:512], wqk_pcn[:, :, 128:512])
            bqk_sb = wts.tile([1, 512], F32R, tag="bqk")
            nc.scalar.dma_start(bqk_sb[:], bqk_d[:])
            wv_sb = wts.tile([128, NCH, 256], BF16, tag="wv")
            nc.scalar.dma_start(wv_sb[:], wv_d.rearrange("(c p) n -> p c n", p=128))
            bv_sb = wts.tile([1, 256], F32R, tag="bv")
            nc.scalar.dma_start(bv_sb[:], bv_d[:])

            ones = constp.tile([1, 512], F32R, tag="ones")
            nc.scalar.dma_start(ones[:], ones_d[:])
            ones128 = constp.tile([128, 64], F32R, tag="ones128")
            nc.scalar.dma_start(ones128[:], ones128_d[:])
            bs_sb = constp.tile([128, NKB], F32, tag="bs")
            nc.scalar.dma_start(bs_sb[:], bs_d[:])

            wo_sb = wts.tile([128, 2, D], F32R, tag="wo")
            nc.scalar.dma_start(wo_sb[:], wo_d.rearrange("(c p) n -> p c n", p=128))

            qkt = [
                bigp.tile([128, T], BF16, tag=f"qkt{db}", name=f"qkt{db}")
                for db in range(4)
            ]
            vsb = bigp.tile([128, NKB, HPC, 65], BF16, tag="vsb", name="vsb_v10")
            nc.gpsimd.memset(vsb[:], 1.0)
            ot = [
                bigp.tile([128, T], F32R, tag=f"ot{pi}", name=f"ot{pi}")
                for pi in range(2)
            ]
            y_rows = y_d.rearrange("(n p) d -> n p d", p=128)

            # ---------------- emission helpers ----------------
            def emit_A(tb):
                for db in range(4):
                    ps = psG.tile([128, 512], F32, tag="gp", name=f"qk{tb}_{db}")
                    for c in range(NCH):
                        nc.tensor.matmul(
                            ps[:],
                            wqk_sb[:, c, db * 128:(db + 1) * 128],
                            xT[:, c, tb * 512:(tb + 1) * 512],
                            start=(c == 0),
                            stop=(not with_qkv_bias and c == NCH - 1),
                        )
                    if with_qkv_bias:
                        nc.tensor.matmul(
                            ps[:],
                            bqk_sb[0:1, db * 128:(db + 1) * 128],
                            ones[0:1, :],
                            start=False,
                            stop=True,
                        )
                    nc.vector.tensor_copy(qkt[db][:, tb * 512:(tb + 1) * 512], ps[:])

            def emit_B(tb):
                for j in range(4):
                    kb = tb * 4 + j
                    ps = psG.tile([128, 256], F32, tag="gp", name=f"v{kb}")
                    for c in range(NCH):
                        nc.tensor.matmul(
                            ps[:],
                            xT[:, c, kb * 128:(kb + 1) * 128],
                            wv_sb[:, c, :],
                            start=(c == 0),
                            stop=(not with_qkv_bias and c == NCH - 1),
                        )
                    if with_qkv_bias:
                        nc.tensor.matmul(
                            ps[:], ones[0:1, 0:128], bv_sb[:], start=False, stop=True
                        )
                    nc.vector.tensor_copy(
                        vsb[:, kb, :, 0:64],
                        ps[:].rearrange("p (h c) -> p h c", h=HPC),
                    )

            def emit_S_exp(qb, pi, kb):
                qdb, kdb = pi, 2 + pi
                s01 = psS.tile([128, 1024], F32, tag="s01", name=f"s{qb}_{pi}_{kb}")
                nc.tensor.matmul(
                    s01[:, 0:512],
                    qkt[kdb][0:64, kb * 128:(kb + 1) * 128],
                    qkt[qdb][0:64, qb * 512:(qb + 1) * 512],
                )
                nc.tensor.matmul(
                    s01[:, 512:1024],
                    qkt[kdb][64:128, kb * 128:(kb + 1) * 128],
                    qkt[qdb][64:128, qb * 512:(qb + 1) * 512],
                )
                p01 = ptp.tile([128, 1024], BF16, tag="p01", name=f"p{qb}_{pi}_{kb}")
                nc.scalar.activation(p01[:], s01[:], EXP, bias=bs_sb[:, kb:kb + 1])
                return p01

            def emit_D(qb, pi, kb, p01, osA, osB):
                nc.tensor.matmul(
                    osA[:], vsb[:, kb, 2 * pi, :], p01[:, 0:512],
                    start=(kb == 0), stop=(kb == NKB - 1),
                )
                nc.tensor.matmul(
                    osB[:], vsb[:, kb, 2 * pi + 1, :], p01[:, 512:1024],
                    start=(kb == 0), stop=(kb == NKB - 1),
                )

            stgq_tiles = {}

            def emit_stage_out(qb, pi, osA, osB):
                # Stage O' out of PSUM immediately so the accumulator banks
                # free for the next group; normalization is deferred and
                # batched per q-block (emit_norm_qb).
                if pi == 0:
                    stgq_tiles[qb] = stgp.tile(
                        [65, 4, 512], F32R, tag="stgq", name=f"stgq{qb}"
                    )
                stgq = stgq_tiles[qb]
                nc.vector.tensor_copy(stgq[:, 2 * pi + 0, :], osA[:])
                nc.vector.tensor_copy(stgq[:, 2 * pi + 1, :], osB[:])

            def make_norm_steps(qb):
                # Normalization as schedulable steps so the PE never waits:
                #  step 0: 4 K=1 matmuls fan each head's raw denominator row
                #          into a 32-partition quadrant of one [128,512] PSUM
                #          tile, then ONE exact DVE reciprocal over all 128
                #          partitions (lane-parallel: ~3.2us, vs 12.9us on a
                #          single-partition row). ScalarE stays exp-only.
                #  steps 1..4: per-head K=1 rebroadcast of the f32r reciprocal
                #          + fused normalize-multiply (+ odd-head SBUF shift).
                stgq = stgq_tiles.pop(qb)
                cols = slice(qb * 512, (qb + 1) * 512)
                dt = normp.tile([128, 512], F32R, tag="dt", name=f"dt{qb}")
                rec = normp.tile([128, 512], F32R, tag="rec", name=f"rec{qb}")

                def s_dma(stgq=stgq, dt=dt):
                    nc.gpsimd.memset(dt[:].bitcast(F32), 1.0)
                    # Fan the 4 heads' denominator rows onto partitions
                    # 0/32/64/96 (tiny partition-moving SBUF DMAs) so ONE
                    # [128,512] DVE reciprocal covers all heads lane-parallel
                    # (~3.2us vs 12.9us on a single-partition row).
                    for j in range(4):
                        eng = nc.gpsimd if j % 2 == 0 else nc.sync
                        eng.dma_start(
                            dt[32 * j:32 * j + 1, :], stgq[64:65, j, :]
                        )

                def s_rec(dt=dt, rec=rec):
                    with nc.allow_low_precision(reason="f32r recip broadcast"):
                        nc.vector.reciprocal(
                            rec[0:97, :].opt(), dt[0:97, :].opt()
                        )

                steps = [s_dma, s_rec]
                for j in range(4):
                    def s_j(qb=qb, stgq=stgq, rec=rec, j=j):
                        pi, parity = divmod(j, 2)
                        bc2 = psG.tile(
                            [64, 512], F32, tag="gp", name=f"bc2_{qb}_{j}"
                        )
                        nc.tensor.matmul(
                            bc2[:], ones128[32 * j:32 * j + 1, 0:64],
                            rec[32 * j:32 * j + 1, :],
                            tile_position=(32 * j, 0),
                        )
                        if parity == 0:
                            nc.vector.tensor_mul(
                                ot[pi][0:64, cols], stgq[0:64, j, :], bc2[0:64, :]
                            )
                        else:
                            stag = normp.tile([64, 512], F32R, tag="stag")
                            nc.vector.tensor_mul(
                                stag[:], stgq[0:64, j, :], bc2[0:64, :]
                            )
                            nc.gpsimd.dma_start(ot[pi][64:128, cols], stag[:])
                    steps.append(s_j)
                return steps

            def make_E_chunks(qb, tail=False):
                # Stage E for one q-block, sliced into 16 small closures so the
                # emission can interleave one chunk per k-iteration of the next
                # attention group (keeps PE dense without starving ScalarE).
                # In the tail there is no next group: accumulate in psS-backed
                # PSUM (idle by then) so E never contends with the final
                # norm's bc2 allocations in psG.
                chunks = []
                for j in range(4):
                    tb = qb * 4 + j
                    state = {}

                    def c0(tb=tb, state=state):
                        state["ysb"] = ypool.tile(
                            [128, D], BF16, tag="ysb", name=f"ysb{tb}"
                        )
                        if tail:
                            pair = psS.tile(
                                [128, 2, 512], F32, tag="s01", name=f"yp{tb}"
                            )
                            state["yps"] = [pair[:, 0, :], pair[:, 1, :]]
                        else:
                            state["yps"] = [
                                psG.tile(
                                    [128, 512], F32, tag="gp",
                                    name=f"yps{tb}_{nb}",
                                )
                                for nb in range(2)
                            ]
                        nc.tensor.matmul(
                            state["yps"][0],
                            ot[0][:, tb * 128:(tb + 1) * 128],
                            wo_sb[:, 0, 0:512],
                            start=True, stop=False,
                        )

                    def c1(tb=tb, state=state):
                        nc.tensor.matmul(
                            state["yps"][1],
                            ot[0][:, tb * 128:(tb + 1) * 128],
                            wo_sb[:, 0, 512:1024],
                            start=True, stop=False,
                        )

                    def c2(tb=tb, state=state):
                        nc.tensor.matmul(
                            state["yps"][0],
                            ot[1][:, tb * 128:(tb + 1) * 128],
                            wo_sb[:, 1, 0:512],
                            start=False, stop=True,
                        )
                        nc.vector.tensor_copy(
                            state["ysb"][:, 0:512], state["yps"][0]
                        )

                    def c3(tb=tb, state=state):
                        nc.tensor.matmul(
                            state["yps"][1],
                            ot[1][:, tb * 128:(tb + 1) * 128],
                            wo_sb[:, 1, 512:1024],
                            start=False, stop=True,
                        )
                        nc.vector.tensor_copy(
                            state["ysb"][:, 512:1024], state["yps"][1]
                        )
                        nc.sync.dma_start(y_rows[tb], state["ysb"][:])

                    chunks += [c0, c1, c2, c3]
                return chunks

            def emit_E(qb, tail=False):
                for ch in make_E_chunks(qb, tail=tail):
                    ch()

            # ---------------- pipelined emission ----------------
            # Phase 1: stages A/B per t-group, with CD(q0, pair0) k-iterations
            # interleaved so ScalarE ramps while the PE grinds projections.
            osA = psO.tile([65, 512], F32, tag="osA", name="osA0_0")
            osB = psO.tile([65, 512], F32, tag="osB", name="osB0_0")
            for tb in range(4):
                emit_A(tb)
                emit_B(tb)
                for kb in range(4 * tb, 4 * tb + 4):
                    p01 = emit_S_exp(0, 0, kb)
                    emit_D(0, 0, kb, p01, osA, osB)
            pending = [(0, 0, osA, osB)]

            # Phase 2: remaining groups; each group's first two S/exp pairs
            # are emitted before the previous group's epilogue so ACT stays fed
            # across the boundary. Norm steps and E chunks of the previous
            # q-block are slotted at fixed k-iterations so their PE pieces
            # never head-of-line-block on DVE results.
            groups = [(0, 1)] + [(qb, pi) for qb in range(1, NQB) for pi in range(2)]
            e_chunks = []
            norm_steps = []
            for qb, pi in groups:
                head = [emit_S_exp(qb, pi, kb) for kb in (0, 1)]
                pqb, ppi, posA, posB = pending.pop()
                emit_stage_out(pqb, ppi, posA, posB)
                if ppi == 1:
                    norm_steps = make_norm_steps(pqb)
                    e_chunks = make_E_chunks(pqb)
                osA = psO.tile([65, 512], F32, tag="osA", name=f"osA{qb}_{pi}")
                osB = psO.tile([65, 512], F32, tag="osB", name=f"osB{qb}_{pi}")
                for kb in (0, 1):
                    emit_D(qb, pi, kb, head[kb], osA, osB)
                for kb in range(2, NKB):
                    p01 = emit_S_exp(qb, pi, kb)
                    emit_D(qb, pi, kb, p01, osA, osB)
                    if norm_steps and kb in (2, 3, 5, 6, 8, 9):
                        norm_steps.pop(0)()
                    elif (not norm_steps and kb >= 10
                          and len(e_chunks) > (6 if (qb, pi) == groups[-1] else 0)):
                        e_chunks.pop(0)()
                        if kb >= 11 and len(e_chunks) > (
                                6 if (qb, pi) == groups[-1] else 0):
                            e_chunks.pop(0)()
                pending = [(qb, pi, osA, osB)]

            qb, pi, osA, osB = pending.pop()
            emit_stage_out(qb, pi, osA, osB)
            for s in make_norm_steps(qb):
                s()
                if e_chunks:
                    e_chunks.pop(0)()
            while e_chunks:
                e_chunks.pop(0)()
            emit_E(qb, tail=True)

    nc.compile()
    return nc


def _get_nc(with_qkv_bias=True):
    key = ("nc", with_qkv_bias)
    if key not in _NC_CACHE:
        _NC_CACHE[key] = _build_nc(with_qkv_bias)
    return _NC_CACHE[key]


def _make_in_maps(x, boundary_score, W_qkv, b_qkv, W_out):
    x = np.asarray(x, np.float32)
    boundary_score = np.asarray(boundary_score, np.float32)
    W_qkv = np.asarray(W_qkv, np.float32)
    b_qkv = np.asarray(b_qkv, np.float32)
    W_out = np.asarray(W_out, np.float32)

    Wq, Wk, Wv = W_qkv[:, :D], W_qkv[:, D:2 * D], W_qkv[:, 2 * D:]
    bq, bk, bv = b_qkv[:D], b_qkv[D:2 * D], b_qkv[2 * D:]
    ones = np.ones((1, 512), np.float32)
    ones128 = np.ones((128, 64), np.float32)
    import ml_dtypes
    bf16 = ml_dtypes.bfloat16
    xts = [np.ascontiguousarray(x[b].T).astype(bf16) for b in range(x.shape[0])]

    in_maps = []
    for c in range(8):
        b, g = divmod(c, 4)
        lo, hi = 256 * g, 256 * (g + 1)
        wqk = np.ascontiguousarray(
            np.concatenate([Wq[:, lo:hi] * SCALE, Wk[:, lo:hi]], axis=1)
        ).astype(bf16)
        bqk = np.concatenate([bq[lo:hi] * SCALE, bk[lo:hi]])[None]
        wv = np.ascontiguousarray(Wv[:, lo:hi]).astype(bf16)
        bvv = np.ascontiguousarray(bv[lo:hi][None])
        wo = np.ascontiguousarray(W_out[lo:hi, :])
        bs = np.ascontiguousarray(
            (boundary_score[b] * BIAS_COEF).reshape(NKB, 128).T
        )
        in_maps.append(
            dict(
                xt=xts[b], wqk=wqk, bqk=np.ascontiguousarray(bqk),
                wv=wv, bv=bvv, wo=wo, bs=bs, ones=ones, ones128=ones128,
            )
        )
    return in_maps


def kernel(x, boundary_score, W_qkv, b_qkv, W_out, b_out):
    from concourse.bass_utils import run_bass_kernel_spmd

    x = np.asarray(x, np.float32)
    B = x.shape[0]
    in_maps = _make_in_maps(x, boundary_score, W_qkv, b_qkv, W_out)
    nc = _get_nc(with_qkv_bias=bool(np.any(np.asarray(b_qkv))))
    res = run_bass_kernel_spmd(nc, in_maps, list(range(8))).results
    out = np.zeros((B, T, D), np.float32)
    for c in range(8):
        out[c // 4] += np.asarray(res[c]["y"], np.float32)
    out += np.asarray(b_out, np.float32)
    return out

